# revision 2
# baseline (speedup 1.0000x reference)
"""GQA attention kernel for Trainium2, tensor-parallel across 8 NeuronCores.

Problem: B=2, T=2048, D=2048, H=32 q-heads, G=8 kv-heads (GQA, rep=4), hd=64,
causal softmax attention + output projection, fp32 I/O.

Sharding (one KV group per core):
  core g: Wq[:, g*256:(g+1)*256], Wk/Wv[:, g*64:(g+1)*64], Wo[g*256:(g+1)*256, :]
  Each core computes its 4 heads' attention + partial output projection;
  host sums the 8 partial outputs (row-parallel Wo => partial-sum unshard).

Changes vs the previous 223us version:
  * Scores in one fp8 DoubleRow matmul per (kt, head) instead of one bf16
    matmul: stationary = [kh; kl] (error-split K, exact to ~0.3%), moving =
    [qh; qh] (single fp8 Q at scale 1/2, ~2.4% quant noise -> ~1.3e-2 final
    rel err, inside the 2e-2 gate). Halves the score PE cost (the 1/64
    unscale is folded into the exp's activation scale).
    K weights are host-scaled by 16 (not 64) so the K psum (16k, |max|~80)
    fits fp8e4's 448 range when split; V keeps WSCALE=64.
  * b=1's projections are emitted inside b=0's attention phase (and early
    b=0 projections inside the early attention blocks) so the exp stream on
    ACT never starves and PE idle phases vanish.
  * Wo psum->SBUF copies alternate DVE/Pool so DVE stays under the PE time.

Per-core dataflow otherwise matches the previous version: fp8 error-split
DoubleRow projections (hi@hi + lo@hi + hi@lo over ko-pairs), V transposed
via PE identity matmuls to v1 [kpos, 16, hd|1] with a ones column, PV with
P-slices stationary (65-cycle matmuls), DVE reciprocal normalize, o_n
transposed by DMA xbar, Wo partial via 3-pass fp8 DoubleRow.
"""

import os
import sys

import numpy as np

for _p in ("/opt/trn_rl_repo", "/root/.axon_site/_ro/trn_rl_repo"):
    if os.path.isdir(_p) and _p not in sys.path:
        sys.path.insert(0, _p)

import ml_dtypes  # noqa: E402

import concourse.bass as bass  # noqa: E402
import concourse.mybir as mybir  # noqa: E402
import concourse.tile as tile  # noqa: E402
from concourse import bacc  # noqa: E402
from concourse.bass_utils import run_bass_kernel_spmd  # noqa: E402
from concourse.masks import make_identity  # noqa: E402
from contextlib import ExitStack  # noqa: E402

B, T, D = 2, 2048, 2048
G, REP, HD = 8, 4, 64
DQ = REP * HD  # 256 q-dims per core
NCORES = 8
P = 128
TB = 512  # q/t block size
KO = D // P  # 16 contraction subtiles for projections
KQ = 4  # ko tiles per x DMA load
NT = T // TB  # 4 t-blocks
NKT = T // P  # 16 kpos tiles
F32 = mybir.dt.float32
BF16 = mybir.dt.bfloat16
FP8 = mybir.dt.float8e4
DR = mybir.MatmulPerfMode.DoubleRow
WSCALE = 64.0  # host multiplies Wq/Wv/Wo by this before fp8 split
WSCALE_K = 16.0  # K columns: smaller so fp8(16k) stays in e4m3 range
SQ = 0.5  # qh = q * SQ in fp8
SEXP = 1.0 / (8.0 * SQ * WSCALE_K)  # exp scale: psum = 8*SQ*WSCALE_K*s... see attn
AF = mybir.ActivationFunctionType
PJ_BUFS = 1
S_BUFS = 2
O_BUFS = 2
W_BUFS = 1
XT_BUFS = 8
XT1_BUFS = 2  # unused
KQ2 = 4  # effective KQ (overrides KQ below for sweeps)
BANDS = True      # use priority bands for attn stream / proj copies
OT8_POOL = True  # put the ot8 fp8 split on Pool instead of DVE
SCHED = "v1"     # emission order variant
DEFER = True      # defer late Wo row-blocks into the exp-bound tail
DRAIN_SPLIT = True  # split final-drain psum copies between ACT and DVE
PROJ_PSUM = "w"
HEAD_PJ = True
DRAIN_OT8_DVE = False
OT8_BUFS = 2
STG_BUFS = 2
WARMUP = 0  # junk PE transposes at t=0 (0 disables)  # "pj": proj chains+tr self-contained on PJ; "w": borrow W


def build_kernel(ctx, tc):
    nc = tc.nc
    from contextlib import contextmanager

    # Priority bands (lower = earlier = higher scheduler priority):
    #   [0, 500k)    attention score->exp stream (the serial ACT bottleneck;
    #                scores must preempt fill work the moment psum frees)
    #   [500k, 1M)   projection psum->SBUF copies + dup DMAs + xt loads
    #                (they gate the NEXT attention block's scores)
    #   [1M, ...)    everything else (proj matmuls, PV, Wo, stores) = fill
    tc.cur_priority = 1_000_000
    _bands = {"attn": [0], "proj": [500_000]}

    @contextmanager
    def band(name):
        if not BANDS:
            yield
            return
        sv = tc.cur_priority
        tc.cur_priority = _bands[name][0]
        try:
            yield
        finally:
            _bands[name][0] = tc.cur_priority
            tc.cur_priority = sv
    xh = nc.dram_tensor("xh", [B, D, T], FP8, kind="ExternalInput").ap()
    xl = nc.dram_tensor("xl", [B, D, T], FP8, kind="ExternalInput").ap()
    wqh = nc.dram_tensor("wqh", [D, DQ], FP8, kind="ExternalInput").ap()
    wql = nc.dram_tensor("wql", [D, DQ], FP8, kind="ExternalInput").ap()
    wkvh = nc.dram_tensor("wkvh", [D, 2 * HD], FP8, kind="ExternalInput").ap()
    wkvl = nc.dram_tensor("wkvl", [D, 2 * HD], FP8, kind="ExternalInput").ap()
    woh = nc.dram_tensor("woh", [DQ, D], FP8, kind="ExternalInput").ap()
    wol = nc.dram_tensor("wol", [DQ, D], FP8, kind="ExternalInput").ap()
    out = nc.dram_tensor("out", [B, T, D], BF16, kind="ExternalOutput").ap()

    wpool = ctx.enter_context(tc.tile_pool(name="w", bufs=1))
    qt_pool = ctx.enter_context(tc.tile_pool(name="qt", bufs=2))
    kkt_pool = ctx.enter_context(tc.tile_pool(name="kkt", bufs=2))
    vt_pool = ctx.enter_context(tc.tile_pool(name="vt", bufs=2))
    v_pool = ctx.enter_context(tc.tile_pool(name="v", bufs=2))
    xt_pool = ctx.enter_context(tc.tile_pool(name="xt", bufs=XT_BUFS))
    p_pool = ctx.enter_context(tc.tile_pool(name="p", bufs=2))
    on_pool = ctx.enter_context(tc.tile_pool(name="on", bufs=3))
    rc_pool = ctx.enter_context(tc.tile_pool(name="rc", bufs=3))
    ot_pool = ctx.enter_context(tc.tile_pool(name="ot", bufs=2))
    ot8_pool = ctx.enter_context(tc.tile_pool(name="ot8", bufs=OT8_BUFS))
    stg_pool = ctx.enter_context(tc.tile_pool(name="stg", bufs=STG_BUFS))
    pp = ctx.enter_context(tc.tile_pool(name="pp", bufs=2, space="PSUM"))

    # persistent weights (SP/HWDGE queue; Pool is reserved for xt loads).
    # wq/wkv split into ko-chunks so the first matmuls wait only on chunk 0.
    wqh_sb = wpool.tile([P, KO, DQ], FP8, tag="wqh")
    wql_sb = wpool.tile([P, KO, DQ], FP8, tag="wql")
    wkvh_sb = wpool.tile([P, KO, 2 * HD], FP8, tag="wkvh")
    wkvl_sb = wpool.tile([P, KO, 2 * HD], FP8, tag="wkvl")
    for sb, dr in ((wqh_sb, wqh), (wkvh_sb, wkvh), (wql_sb, wql), (wkvl_sb, wkvl)):
        r = dr.rearrange("(ko p) m -> p ko m", p=P)
        for c in range(0, KO, KQ):
            nc.sync.dma_start(sb[:, c : c + KQ, :], r[:, c : c + KQ, :])
    woh_sb = wpool.tile([P, DQ // P, D], FP8, tag="woh")
    nc.sync.dma_start(woh_sb[:], woh.rearrange("(ko p) m -> p ko m", p=P))
    wol_sb = wpool.tile([P, DQ // P, D], FP8, tag="wol")
    nc.sync.dma_start(wol_sb[:], wol.rearrange("(ko p) m -> p ko m", p=P))
    # upper-triangular causal mask (keep f >= p), two identical copies so one
    # tensor_tensor covers both head halves of a pair at once
    ident = wpool.tile([HD, HD], BF16, tag="ident")
    make_identity(nc, ident[:])
    ident128 = wpool.tile([P, P], BF16, tag="id128")
    make_identity(nc, ident128[:])
    tri = wpool.tile([P, 2, P], BF16, tag="tri")
    nc.gpsimd.memset(tri[:], 1.0)
    for h in range(2):
        nc.gpsimd.affine_select(
            out=tri[:, h, :],
            in_=tri[:, h, :],
            compare_op=mybir.AluOpType.is_ge,
            fill=0.0,
            base=0,
            channel_multiplier=-1,
            pattern=[[1, P]],
        )

    def batch_state(b):
        st = {"b": b}
        # qt8: fp8 Q at scale SQ; dims [part(2 heads x 64), pair, T]. The
        # score DR matmul reads it through a stride-0 broadcast AP, so no
        # physical duplicate is needed.
        st["qt8"] = qt_pool.tile([P, 2, T], FP8, tag="qt", name=f"qt8_{b}")
        # khl: fp8 split of 16k; parts 0:64 = (kh, kl), 64:128 = duplicate
        st["khl"] = kkt_pool.tile([P, 2, T], FP8, tag="khl", name=f"khl_{b}")

        st["v1"] = v_pool.tile([P, NKT, HD + 1], BF16, tag="v1", name=f"v1_{b}")
        nc.gpsimd.memset(st["v1"][:, :, HD : HD + 1], 1.0)
        return st

    def proj(st, tb):
        # ---------------- projections for t-block tb ----------------
        b = st["b"]
        qt8_sb, khl_sb, v1_sb = st["qt8"], st["khl"], st["v1"]
        vt_sb = vt_pool.tile([HD, TB], BF16, tag="vt", name="vt")
        ts = slice(tb * TB, (tb + 1) * TB)
        xhs, xls = [], []
        with band("proj"):
            for src_t, lst, tag in ((xh, xhs, "xh"), (xl, xls, "xl")):
                for kq in range(KO // KQ):
                    xt = xt_pool.tile(
                        [P, KQ, TB], FP8, tag=tag, name="xt", bufs=XT_BUFS,
                    )
                    nc.gpsimd.dma_start(
                        xt[:],
                        src_t[b, kq * KQ * P : (kq + 1) * KQ * P, ts].rearrange(
                            "(q p) t -> p q t", p=P
                        ),
                    )
                    lst.append(xt)
        # three sequential accumulation chains (Q pair0, Q pair1, KV), each
        # as 3 fp8 DoubleRow passes (hi@hi + lo@hi + hi@lo) over ko-pairs.
        for ci, (whsb, wlsb, lo) in (
            (0, (wqh_sb, wql_sb, 0)),
            (2, (wkvh_sb, wkvl_sb, 0)),
            (1, (wqh_sb, wql_sb, P)),
        ):
            if b == 0 and tb == 0 and ci == 2:
                c_ps = pp.tile([P, TB], F32, tag="O", bufs=O_BUFS)
            elif PROJ_PSUM == "w" and b == 0 and tb >= 1 and ci == 1:
                c_ps = pp.tile([P, TB], F32, tag="W", bufs=W_BUFS)
            elif b == 0 and tb == 0 and ci < (1 if HEAD_PJ else 2):
                # before any attention exists the score psum is idle:
                # borrow S slots so the first three chains overlap
                sbig = pp.tile(
                    [P, 2, TB], F32, tag="S", bufs=S_BUFS, name=f"sb{ci}"
                )
                c_ps = sbig[:, 0]
            else:
                c_ps = pp.tile([P, TB], F32, tag="PJ", bufs=PJ_BUFS)
            passes = ((whsb, xhs), (whsb, xls), (wlsb, xhs))
            n_mm = len(passes) * (KO // 2)
            i = 0
            for wsb, xlist in passes:
                for kp in range(KO // 2):
                    ko = 2 * kp
                    nc.tensor.matmul(
                        c_ps[:],
                        wsb[:, ko : ko + 2, lo : lo + P],
                        xlist[ko // KQ][:, ko % KQ : ko % KQ + 2, :],
                        start=(i == 0),
                        stop=(i == n_mm - 1),
                        perf_mode=DR,
                    )
                    i += 1
            with band("proj"):
                if ci < 2:
                    # qh = q * SQ in fp8 (psum holds 64q)
                    nc.vector.tensor_scalar_mul(
                        qt8_sb[:, ci, ts], c_ps[:], SQ / WSCALE
                    )
                else:
                    # K psum holds 16k (host scaled Wk by 16): split to fp8
                    nc.vector.tensor_copy(khl_sb[0:HD, 0, ts], c_ps[0:HD, :])
                    nc.vector.tensor_tensor(
                        out=khl_sb[0:HD, 1, ts],
                        in0=c_ps[0:HD, :],
                        in1=khl_sb[0:HD, 0, ts],
                        op=mybir.AluOpType.subtract,
                    )
                    nc.vector.tensor_scalar_mul(
                        vt_sb[:], c_ps[HD:P, :], 1.0 / WSCALE
                    )
        # duplicate khl to partitions 64..127 (SBUF->SBUF DMA) so each
        # head-half's DR matmul has its stationary on its own partitions
        with band("proj"):
            nc.sync.dma_start(khl_sb[HD:P, :, ts], khl_sb[0:HD, :, ts])
        # V transpose via PE identity matmul: [64, 128] -> [128, 64]
        if PROJ_PSUM == "w" and b == 0:
            tr_ps = pp.tile([P, 4, HD], BF16, tag="W", bufs=W_BUFS, name="trw")
        else:
            tr_ps = pp.tile([P, 4, HD], BF16, tag="PJ", bufs=PJ_BUFS, name="tr")
        for i in range(4):
            nc.tensor.transpose(
                tr_ps[:, i], vt_sb[:, i * P : (i + 1) * P], ident[:]
            )
        with band("proj"):
            nc.vector.tensor_copy(v1_sb[:, 4 * tb : 4 * tb + 4, 0:HD], tr_ps[:])

    def attn(st, qb):
        # ------------- attention scores + exp for q-block qb -----------
        b = st["b"]
        qt8_sb, khl_sb = st["qt8"], st["khl"]
        nkt = 4 * (qb + 1)  # causal: kpos tiles 0..nkt-1
        p4 = p_pool.tile([P, 2, 2, nkt, TB], BF16, tag=f"P{qb % 2}", bufs=1)
        with band("attn"):
            for kt in range(nkt):
                for pair in range(2):
                    p_sb = p4[:, pair]
                    ks = slice(kt * P, (kt + 1) * P)
                    dk = kt - qb * 4
                    off = max(dk, 0) * P  # first potentially-valid column
                    s_ps = pp.tile([P, 2, TB], F32, tag="S", bufs=S_BUFS)
                    qs = slice(qb * TB + off, (qb + 1) * TB)
                    # one fp8 DR matmul per head: (kh,kl) stationary x
                    # (qh,qh) moving = k . qh, K exact; psum = 8*score.
                    # The moving (qh,qh) pair is a stride-0 broadcast AP.
                    for half in range(2):
                        hp = slice(HD * half, HD * (half + 1))
                        qmv = qt8_sb[hp, pair, qs].unsqueeze(1)
                        nc.tensor.matmul(
                            s_ps[:, half, off:],
                            khl_sb[hp, :, ks],
                            qmv.broadcast_to([HD, 2, qmv.shape[2]]),
                            start=True,
                            stop=True,
                            perf_mode=DR,
                        )
                    nc.scalar.activation(
                        p_sb[:, :, kt, off:], s_ps[:, :, off:], AF.Exp,
                        scale=SEXP,
                    )
                    if dk >= 0:  # diagonal block: causal triangle mask
                        nc.vector.tensor_mul(
                            p_sb[:, :, kt, off : off + P],
                            p_sb[:, :, kt, off : off + P],
                            tri[:],
                        )
        return p4

    def attn_b(st, qb, p4):
        b = st["b"]
        v1_sb = st["v1"]
        ot_sb = ot_pool.tile([P, 2, TB], BF16, tag="ot")
        ot8h = ot8_pool.tile([P, 2, TB], FP8, tag="oh")
        ot8l = ot8_pool.tile([P, 2, TB], FP8, tag="ol")
        # --- phase B: PV accumulation, normalize, transpose ---
        def bj(j):
            for pair in range(2):
                p_sb = p4[:, pair]
                ktn = qb * 4 + j + 1  # kpos tiles 0..ktn-1
                o_n = on_pool.tile([P, 2, HD], BF16, tag="on")
                for half in range(2):
                    o_ps = pp.tile([P, HD + 1], F32, tag="O", bufs=O_BUFS)
                    for kt in range(ktn):
                        nc.tensor.matmul(
                            o_ps[:],
                            p_sb[:, half, kt, j * P : (j + 1) * P],
                            v1_sb[:, kt, :],
                            start=(kt == 0),
                            stop=(kt == ktn - 1),
                        )
                    rec = rc_pool.tile([P, 1], F32, tag="rec")
                    nc.vector.reciprocal(rec[:], o_ps[:, HD : HD + 1])
                    nc.vector.tensor_scalar(
                        o_n[:, half, :], o_ps[:, 0:HD], rec[:], 8.0,
                        mybir.AluOpType.mult, mybir.AluOpType.mult,
                    )
                js = slice(j * P, (j + 1) * P)
                if b == B - 1 and qb == 3 and j == 3:
                    # drain path: PE transpose instead of the xbar DMA
                    # transpose (~1.7 us latency); S is idle by then
                    tp = pp.tile(
                        [P, 2, TB], BF16, tag="S", bufs=S_BUFS,
                        name=f"tp{pair}",
                    )
                    nc.tensor.transpose(tp[:, 0, 0:P], o_n[:], ident128[:])
                    nc.vector.tensor_copy(ot_sb[:, pair, js], tp[:, 0, 0:P])
                else:
                    nc.sync.dma_start_transpose(ot_sb[:, pair, js], o_n[:])
                # SBUF-only fp8 split of ot (the piece of DVE work that
                # CAN move to Pool; GPSIMD cannot touch PSUM). The final
                # drain block stays on DVE for latency.
                drainb = DRAIN_OT8_DVE and b == B - 1 and qb == 3
                e8 = nc.gpsimd if (OT8_POOL and not drainb) else nc.vector
                e8.tensor_copy(ot8h[:, pair, js], ot_sb[:, pair, js])
                e8.tensor_sub(
                    ot8l[:, pair, js], ot_sb[:, pair, js], ot8h[:, pair, js]
                )
        return bj, (ot8h, ot8l)

    def _wo_j(st, qb, ots, j, split_stores=False):
        b = st["b"]
        ot8h, ot8l = ots
        rows = slice(qb * TB + j * P, qb * TB + (j + 1) * P)
        stg = stg_pool.tile([P, D], BF16, tag="stg")
        for nb in range(4):
            if b == 1 and qb == 2 and nb in (1, 3):
                wo_ps = pp.tile(
                    [P, TB], F32, tag="PJ", bufs=PJ_BUFS,
                    name=f"w2P{j}{nb}",
                )
            elif split_stores:
                # final block: every other psum tag is idle by now --
                # rotate through them so Wo is not W-recycle-bound
                if nb == 3 or nb == 1:
                    wo_ps = pp.tile(
                        [P, 2, TB], F32, tag="S", bufs=S_BUFS,
                        name=f"wS{j}{nb}",
                    )[:, 0]
                elif nb == 2:
                    wo_ps = pp.tile(
                        [P, TB], F32, tag="PJ", bufs=PJ_BUFS,
                        name=f"wP{j}",
                    )
                else:
                    wo_ps = pp.tile([P, TB], F32, tag="W", bufs=W_BUFS)
            else:
                wo_ps = pp.tile([P, TB], F32, tag="W", bufs=W_BUFS)
            ns = slice(nb * TB, (nb + 1) * TB)
            for i, (osb, wsb) in enumerate(
                ((ot8h, woh_sb), (ot8l, woh_sb), (ot8h, wol_sb))
            ):
                nc.tensor.matmul(
                    wo_ps[:],
                    osb[:, :, j * P : (j + 1) * P],
                    wsb[:, :, ns],
                    start=(i == 0),
                    stop=(i == 2),
                    perf_mode=DR,
                )
            if split_stores and (nb % 2 == 0 or not DRAIN_SPLIT):
                # final block: ACT is mostly idle (no exps left)
                nc.scalar.mul(stg[:, ns], wo_ps[:], 1.0 / (8.0 * WSCALE))
            else:
                nc.vector.tensor_scalar_mul(
                    stg[:, ns], wo_ps[:], 1.0 / (8.0 * WSCALE)
                )
            if split_stores:
                nc.sync.dma_start(
                    out[b, rows, nb * TB : (nb + 1) * TB],
                    stg[:, nb * TB : (nb + 1) * TB],
                )
        if not split_stores:
            nc.sync.dma_start(out[b, rows, :], stg[:])

    def ab_wo(st, qb, p4, split_stores=False, defer_wo_js=()):
        # Phase B + Wo, j-major so each row-block's Wo unlocks early.
        # defer_wo_js postpones those row-blocks' Wo to the returned thunk
        # (emitted later = lower priority = fills the exp-bound tail).
        bj, ots = attn_b(st, qb, p4)
        deferred = []
        for j in range(NT):
            bj(j)
            if j in defer_wo_js:
                deferred.append(j)
            else:
                _wo_j(st, qb, ots, j, split_stores)

        def run_deferred():
            for j in deferred:
                _wo_j(st, qb, ots, j, split_stores)

        return run_deferred

    # ---- global schedule. Emission order = scheduler priority. Rules:
    # * each proj is emitted BEFORE the neighboring ab_wo so its psum->SBUF
    #   copies (which gate the next attention block's scores) outrank the
    #   ab_wo normalize/Wo copies on the shared DVE queue;
    # * attn(qb+1) right before/after ab_wo(qb) as in the exp-stream
    #   pipeline; a P-tag is only reused after its readers are emitted;
    # * b1 projections fill b0's ACT-bound qb3 window; late Wo row-blocks
    #   are deferred into the underfilled b1-qb2/qb3 windows (each deferral
    #   lands before the ot8 buffer (bufs=2) it reads is recycled). ----
    s0 = batch_state(0)
    s1 = batch_state(1)
    if SCHED == "v5":
        proj(s0, 0)
        proj(s0, 1)
        p1 = attn(s0, 0)
        proj(s0, 2)
        p2 = attn(s0, 1)
        ab_wo(s0, 0, p1)
        proj(s0, 3)
        p3 = attn(s0, 2)
        ab_wo(s0, 1, p2)
        proj(s1, 0)
        p4_ = attn(s0, 3)
        ab_wo(s0, 2, p3)
        proj(s1, 1)
        d03 = ab_wo(s0, 3, p4_, defer_wo_js=(1, 2, 3))
        proj(s1, 2)
        proj(s1, 3)
        q1 = attn(s1, 0)
        q2 = attn(s1, 1)
        d10 = ab_wo(s1, 0, q1, defer_wo_js=(2, 3))
        d03()
        q3 = attn(s1, 2)
        d11 = ab_wo(s1, 1, q2, defer_wo_js=(2, 3))
        d10()
        q4_ = attn(s1, 3)
        d12 = ab_wo(s1, 2, q3, defer_wo_js=(2, 3))
        d11()
        d12()
        ab_wo(s1, 3, q4_, split_stores=True)
    elif SCHED == "v4":
        proj(s0, 0)
        proj(s0, 1)
        p1 = attn(s0, 0)
        proj(s0, 2)
        p2 = attn(s0, 1)
        ab_wo(s0, 0, p1)
        proj(s0, 3)
        p3 = attn(s0, 2)
        proj(s1, 0)
        ab_wo(s0, 1, p2)
        proj(s1, 1)
        p4_ = attn(s0, 3)
        ab_wo(s0, 2, p3)
        proj(s1, 2)
        proj(s1, 3)
        ab_wo(s0, 3, p4_)
        q1 = attn(s1, 0)
        q2 = attn(s1, 1)
        ab_wo(s1, 0, q1)
        q3 = attn(s1, 2)
        ab_wo(s1, 1, q2)
        q4_ = attn(s1, 3)
        dfr = ab_wo(s1, 2, q3, defer_wo_js=(2, 3) if DEFER else ())
        dfr()
        ab_wo(s1, 3, q4_, split_stores=True)
    elif SCHED == "v1":
        proj(s0, 0)
        proj(s0, 1)
        p1 = attn(s0, 0)
        proj(s0, 2)
        p2 = attn(s0, 1)
        ab_wo(s0, 0, p1)
        proj(s0, 3)
        p3 = attn(s0, 2)
        ab_wo(s0, 1, p2)
        proj(s1, 0)
        p4_ = attn(s0, 3)
        ab_wo(s0, 2, p3)
        proj(s1, 1)
        ab_wo(s0, 3, p4_)
        proj(s1, 2)
        proj(s1, 3)
        q1 = attn(s1, 0)
        q2 = attn(s1, 1)
        ab_wo(s1, 0, q1)
        q3 = attn(s1, 2)
        ab_wo(s1, 1, q2)
        q4_ = attn(s1, 3)
        dfr = ab_wo(s1, 2, q3, defer_wo_js=(2, 3) if DEFER else ())
        dfr()
        ab_wo(s1, 3, q4_, split_stores=True)
    else:  # v3
        proj(s0, 0)
        p1 = attn(s0, 0)
        proj(s0, 1)
        p2 = attn(s0, 1)
        proj(s0, 2)
        ab_wo(s0, 0, p1)
        p3 = attn(s0, 2)
        proj(s0, 3)
        ab_wo(s0, 1, p2)
        proj(s1, 0)
        p4_ = attn(s0, 3)
        proj(s1, 1)
        ab_wo(s0, 2, p3)
        proj(s1, 2)
        q1 = attn(s1, 0)
        d03 = ab_wo(s0, 3, p4_, defer_wo_js=(2, 3))
        q2 = attn(s1, 1)
        proj(s1, 3)
        d10 = ab_wo(s1, 0, q1, defer_wo_js=(2, 3))
        d03()
        q3 = attn(s1, 2)
        d11 = ab_wo(s1, 1, q2, defer_wo_js=(1, 2, 3))
        d10()
        q4_ = attn(s1, 3)
        d12 = ab_wo(s1, 2, q3, defer_wo_js=(1, 2, 3))
        d11()
        ab_wo(s1, 3, q4_, split_stores=True)
        d12()


_NC_CACHE = {}


def get_nc():
    if "nc" not in _NC_CACHE:
        nc = bacc.Bacc("TRN2", target_bir_lowering=False, debug=False)
        with tile.TileContext(nc) as tc, ExitStack() as ctx:
            build_kernel(ctx, tc)
        nc.compile()
        _NC_CACHE["nc"] = nc
    return _NC_CACHE["nc"]


def make_in_maps(x, Wq, Wk, Wv, Wo):
    FP8NP = ml_dtypes.float8_e4m3

    def fp8_split(a):
        hi = a.astype(FP8NP)
        lo = (a - hi.astype(np.float32)).astype(FP8NP)
        return hi, lo

    xT = np.ascontiguousarray(np.transpose(np.asarray(x, np.float32), (0, 2, 1)))
    xh, xl = fp8_split(xT)
    Wq, Wk, Wv, Wo = (np.asarray(w, np.float32) for w in (Wq, Wk, Wv, Wo))
    in_maps = []
    for g in range(NCORES):
        in_maps.append(
            {
                "xh": xh,
                "xl": xl,
                **dict(
                    zip(
                        ("wqh", "wql"),
                        fp8_split(
                            WSCALE * np.ascontiguousarray(Wq[:, g * DQ : (g + 1) * DQ])
                        ),
                    )
                ),
                **dict(
                    zip(
                        ("wkvh", "wkvl"),
                        fp8_split(
                            np.ascontiguousarray(
                                np.concatenate(
                                    [
                                        WSCALE_K * Wk[:, g * HD : (g + 1) * HD],
                                        WSCALE * Wv[:, g * HD : (g + 1) * HD],
                                    ],
                                    axis=1,
                                )
                            )
                        ),
                    )
                ),
                **dict(
                    zip(
                        ("woh", "wol"),
                        fp8_split(
                            WSCALE
                            * np.ascontiguousarray(Wo[g * DQ : (g + 1) * DQ, :])
                        ),
                    )
                ),
            }
        )
    return in_maps


def run(x, Wq, Wk, Wv, Wo, trace=False):
    nc = get_nc()
    in_maps = make_in_maps(x, Wq, Wk, Wv, Wo)
    res = run_bass_kernel_spmd(nc, in_maps, list(range(NCORES)), trace=trace)
    acc = np.zeros((B, T, D), np.float32)
    for r in res.results:
        acc += np.asarray(r["out"], dtype=np.float32)
    return acc, res


def kernel(x, Wq, Wk, Wv, Wo):
    return run(x, Wq, Wk, Wv, Wo)[0]


# revision 3
# speedup vs baseline: 1.0033x; 1.0033x over previous
"""GQA attention kernel for Trainium2, tensor-parallel across 8 NeuronCores.

Problem: B=2, T=2048, D=2048, H=32 q-heads, G=8 kv-heads (GQA, rep=4), hd=64,
causal softmax attention + output projection, fp32 I/O.

Sharding (one KV group per core):
  core g: Wq[:, g*256:(g+1)*256], Wk/Wv[:, g*64:(g+1)*64], Wo[g*256:(g+1)*256, :]
  Each core computes its 4 heads' attention + partial output projection;
  host sums the 8 partial outputs (row-parallel Wo => partial-sum unshard).

Changes vs the previous 223us version:
  * Scores in one fp8 DoubleRow matmul per (kt, head) instead of one bf16
    matmul: stationary = [kh; kl] (error-split K, exact to ~0.3%), moving =
    [qh; qh] (single fp8 Q at scale 1/2, ~2.4% quant noise -> ~1.3e-2 final
    rel err, inside the 2e-2 gate). Halves the score PE cost (the 1/64
    unscale is folded into the exp's activation scale).
    K weights are host-scaled by 16 (not 64) so the K psum (16k, |max|~80)
    fits fp8e4's 448 range when split; V keeps WSCALE=64.
  * b=1's projections are emitted inside b=0's attention phase (and early
    b=0 projections inside the early attention blocks) so the exp stream on
    ACT never starves and PE idle phases vanish.
  * Wo psum->SBUF copies alternate DVE/Pool so DVE stays under the PE time.

Per-core dataflow otherwise matches the previous version: fp8 error-split
DoubleRow projections (hi@hi + lo@hi + hi@lo over ko-pairs), V transposed
via PE identity matmuls to v1 [kpos, 16, hd|1] with a ones column, PV with
P-slices stationary (65-cycle matmuls), DVE reciprocal normalize, o_n
transposed by DMA xbar, Wo partial via 3-pass fp8 DoubleRow.
"""

import os
import sys

import numpy as np

for _p in ("/opt/trn_rl_repo", "/root/.axon_site/_ro/trn_rl_repo"):
    if os.path.isdir(_p) and _p not in sys.path:
        sys.path.insert(0, _p)

import ml_dtypes  # noqa: E402

import concourse.bass as bass  # noqa: E402
import concourse.mybir as mybir  # noqa: E402
import concourse.tile as tile  # noqa: E402
from concourse import bacc  # noqa: E402
from concourse.bass_utils import run_bass_kernel_spmd  # noqa: E402
from concourse.masks import make_identity  # noqa: E402
from contextlib import ExitStack  # noqa: E402

B, T, D = 2, 2048, 2048
G, REP, HD = 8, 4, 64
DQ = REP * HD  # 256 q-dims per core
NCORES = 8
P = 128
TB = 512  # q/t block size
KO = D // P  # 16 contraction subtiles for projections
KQ = 4  # ko tiles per x DMA load
NT = T // TB  # 4 t-blocks
NKT = T // P  # 16 kpos tiles
F32 = mybir.dt.float32
BF16 = mybir.dt.bfloat16
FP8 = mybir.dt.float8e4
DR = mybir.MatmulPerfMode.DoubleRow
WSCALE = 64.0  # host multiplies Wq/Wv/Wo by this before fp8 split
WSCALE_K = 16.0  # K columns: smaller so fp8(16k) stays in e4m3 range
SQ = 0.5  # qh = q * SQ in fp8
SEXP = 1.0 / (8.0 * SQ * WSCALE_K)  # exp scale: psum = 8*SQ*WSCALE_K*s... see attn
AF = mybir.ActivationFunctionType
PJ_BUFS = 1
S_BUFS = 2
O_BUFS = 2
W_BUFS = 1
XT_BUFS = 8
XT1_BUFS = 2  # unused
KQ2 = 4  # effective KQ (overrides KQ below for sweeps)
BANDS = True      # use priority bands for attn stream / proj copies
OT8_POOL = True  # put the ot8 fp8 split on Pool instead of DVE
SCHED = "v1"     # emission order variant
DEFER = True      # defer late Wo row-blocks into the exp-bound tail
DRAIN_SPLIT = True  # split final-drain psum copies between ACT and DVE
PROJ_PSUM = "w"
HEAD_PJ = True
DRAIN_OT8_DVE = False
OT8_BUFS = 2
STG_BUFS = 2
WARMUP = 0  # junk PE transposes at t=0 (0 disables)
PV_BAND = False  # give PV matmuls a band above generic fill
MASK_POOL = True  # causal mask multiply on Pool instead of DVE  # "pj": proj chains+tr self-contained on PJ; "w": borrow W


def build_kernel(ctx, tc):
    nc = tc.nc
    from contextlib import contextmanager

    # Priority bands (lower = earlier = higher scheduler priority):
    #   [0, 500k)    attention score->exp stream (the serial ACT bottleneck;
    #                scores must preempt fill work the moment psum frees)
    #   [500k, 1M)   projection psum->SBUF copies + dup DMAs + xt loads
    #                (they gate the NEXT attention block's scores)
    #   [1M, ...)    everything else (proj matmuls, PV, Wo, stores) = fill
    tc.cur_priority = 1_000_000
    _bands = {"attn": [0], "proj": [500_000], "pv": [800_000]}

    @contextmanager
    def _null():
        yield

    @contextmanager
    def band(name):
        if not BANDS:
            yield
            return
        sv = tc.cur_priority
        tc.cur_priority = _bands[name][0]
        try:
            yield
        finally:
            _bands[name][0] = tc.cur_priority
            tc.cur_priority = sv
    xh = nc.dram_tensor("xh", [B, D, T], FP8, kind="ExternalInput").ap()
    xl = nc.dram_tensor("xl", [B, D, T], FP8, kind="ExternalInput").ap()
    wqh = nc.dram_tensor("wqh", [D, DQ], FP8, kind="ExternalInput").ap()
    wql = nc.dram_tensor("wql", [D, DQ], FP8, kind="ExternalInput").ap()
    wkvh = nc.dram_tensor("wkvh", [D, 2 * HD], FP8, kind="ExternalInput").ap()
    wkvl = nc.dram_tensor("wkvl", [D, 2 * HD], FP8, kind="ExternalInput").ap()
    woh = nc.dram_tensor("woh", [DQ, D], FP8, kind="ExternalInput").ap()
    wol = nc.dram_tensor("wol", [DQ, D], FP8, kind="ExternalInput").ap()
    out = nc.dram_tensor("out", [B, T, D], BF16, kind="ExternalOutput").ap()

    wpool = ctx.enter_context(tc.tile_pool(name="w", bufs=1))
    qt_pool = ctx.enter_context(tc.tile_pool(name="qt", bufs=2))
    kkt_pool = ctx.enter_context(tc.tile_pool(name="kkt", bufs=2))
    vt_pool = ctx.enter_context(tc.tile_pool(name="vt", bufs=2))
    v_pool = ctx.enter_context(tc.tile_pool(name="v", bufs=2))
    xt_pool = ctx.enter_context(tc.tile_pool(name="xt", bufs=XT_BUFS))
    p_pool = ctx.enter_context(tc.tile_pool(name="p", bufs=2))
    on_pool = ctx.enter_context(tc.tile_pool(name="on", bufs=3))
    rc_pool = ctx.enter_context(tc.tile_pool(name="rc", bufs=3))
    ot_pool = ctx.enter_context(tc.tile_pool(name="ot", bufs=2))
    ot8_pool = ctx.enter_context(tc.tile_pool(name="ot8", bufs=OT8_BUFS))
    stg_pool = ctx.enter_context(tc.tile_pool(name="stg", bufs=STG_BUFS))
    pp = ctx.enter_context(tc.tile_pool(name="pp", bufs=2, space="PSUM"))

    # persistent weights (SP/HWDGE queue; Pool is reserved for xt loads).
    # wq/wkv split into ko-chunks so the first matmuls wait only on chunk 0.
    wqh_sb = wpool.tile([P, KO, DQ], FP8, tag="wqh")
    wql_sb = wpool.tile([P, KO, DQ], FP8, tag="wql")
    wkvh_sb = wpool.tile([P, KO, 2 * HD], FP8, tag="wkvh")
    wkvl_sb = wpool.tile([P, KO, 2 * HD], FP8, tag="wkvl")
    for sb, dr in ((wqh_sb, wqh), (wkvh_sb, wkvh), (wql_sb, wql), (wkvl_sb, wkvl)):
        r = dr.rearrange("(ko p) m -> p ko m", p=P)
        for c in range(0, KO, KQ):
            nc.sync.dma_start(sb[:, c : c + KQ, :], r[:, c : c + KQ, :])
    woh_sb = wpool.tile([P, DQ // P, D], FP8, tag="woh")
    nc.sync.dma_start(woh_sb[:], woh.rearrange("(ko p) m -> p ko m", p=P))
    wol_sb = wpool.tile([P, DQ // P, D], FP8, tag="wol")
    nc.sync.dma_start(wol_sb[:], wol.rearrange("(ko p) m -> p ko m", p=P))
    # upper-triangular causal mask (keep f >= p), two identical copies so one
    # tensor_tensor covers both head halves of a pair at once
    ident = wpool.tile([HD, HD], BF16, tag="ident")
    make_identity(nc, ident[:])
    ident128 = wpool.tile([P, P], BF16, tag="id128")
    make_identity(nc, ident128[:])
    tri = wpool.tile([P, 2, P], BF16, tag="tri")
    nc.gpsimd.memset(tri[:], 1.0)
    for h in range(2):
        nc.gpsimd.affine_select(
            out=tri[:, h, :],
            in_=tri[:, h, :],
            compare_op=mybir.AluOpType.is_ge,
            fill=0.0,
            base=0,
            channel_multiplier=-1,
            pattern=[[1, P]],
        )

    def batch_state(b):
        st = {"b": b}
        # qt8: fp8 Q at scale SQ; dims [part(2 heads x 64), pair, T]. The
        # score DR matmul reads it through a stride-0 broadcast AP, so no
        # physical duplicate is needed.
        st["qt8"] = qt_pool.tile([P, 2, T], FP8, tag="qt", name=f"qt8_{b}")
        # khl: fp8 split of 16k; parts 0:64 = (kh, kl), 64:128 = duplicate
        st["khl"] = kkt_pool.tile([P, 2, T], FP8, tag="khl", name=f"khl_{b}")

        st["v1"] = v_pool.tile([P, NKT, HD + 1], BF16, tag="v1", name=f"v1_{b}")
        nc.gpsimd.memset(st["v1"][:, :, HD : HD + 1], 1.0)
        return st

    def proj(st, tb):
        # ---------------- projections for t-block tb ----------------
        b = st["b"]
        qt8_sb, khl_sb, v1_sb = st["qt8"], st["khl"], st["v1"]
        vt_sb = vt_pool.tile([HD, TB], BF16, tag="vt", name="vt")
        ts = slice(tb * TB, (tb + 1) * TB)
        xhs, xls = [], []
        with band("proj"):
            for src_t, lst, tag in ((xh, xhs, "xh"), (xl, xls, "xl")):
                for kq in range(KO // KQ):
                    xt = xt_pool.tile(
                        [P, KQ, TB], FP8, tag=tag, name="xt", bufs=XT_BUFS,
                    )
                    nc.gpsimd.dma_start(
                        xt[:],
                        src_t[b, kq * KQ * P : (kq + 1) * KQ * P, ts].rearrange(
                            "(q p) t -> p q t", p=P
                        ),
                    )
                    lst.append(xt)
        # three sequential accumulation chains (Q pair0, Q pair1, KV), each
        # as 3 fp8 DoubleRow passes (hi@hi + lo@hi + hi@lo) over ko-pairs.
        for ci, (whsb, wlsb, lo) in (
            (0, (wqh_sb, wql_sb, 0)),
            (2, (wkvh_sb, wkvl_sb, 0)),
            (1, (wqh_sb, wql_sb, P)),
        ):
            if b == 0 and tb == 0 and ci == 2:
                c_ps = pp.tile([P, TB], F32, tag="O", bufs=O_BUFS)
            elif PROJ_PSUM == "w" and b == 0 and tb >= 1 and ci == 1:
                c_ps = pp.tile([P, TB], F32, tag="W", bufs=W_BUFS)
            elif b == 0 and tb == 0 and ci < (1 if HEAD_PJ else 2):
                # before any attention exists the score psum is idle:
                # borrow S slots so the first three chains overlap
                sbig = pp.tile(
                    [P, 2, TB], F32, tag="S", bufs=S_BUFS, name=f"sb{ci}"
                )
                c_ps = sbig[:, 0]
            else:
                c_ps = pp.tile([P, TB], F32, tag="PJ", bufs=PJ_BUFS)
            passes = ((whsb, xhs), (whsb, xls), (wlsb, xhs))
            n_mm = len(passes) * (KO // 2)
            i = 0
            for wsb, xlist in passes:
                for kp in range(KO // 2):
                    ko = 2 * kp
                    nc.tensor.matmul(
                        c_ps[:],
                        wsb[:, ko : ko + 2, lo : lo + P],
                        xlist[ko // KQ][:, ko % KQ : ko % KQ + 2, :],
                        start=(i == 0),
                        stop=(i == n_mm - 1),
                        perf_mode=DR,
                    )
                    i += 1
            with band("proj"):
                if ci < 2:
                    # qh = q * SQ in fp8 (psum holds 64q)
                    nc.vector.tensor_scalar_mul(
                        qt8_sb[:, ci, ts], c_ps[:], SQ / WSCALE
                    )
                else:
                    # K psum holds 16k (host scaled Wk by 16): split to fp8
                    nc.vector.tensor_copy(khl_sb[0:HD, 0, ts], c_ps[0:HD, :])
                    nc.vector.tensor_tensor(
                        out=khl_sb[0:HD, 1, ts],
                        in0=c_ps[0:HD, :],
                        in1=khl_sb[0:HD, 0, ts],
                        op=mybir.AluOpType.subtract,
                    )
                    nc.vector.tensor_scalar_mul(
                        vt_sb[:], c_ps[HD:P, :], 1.0 / WSCALE
                    )
        # duplicate khl to partitions 64..127 (SBUF->SBUF DMA) so each
        # head-half's DR matmul has its stationary on its own partitions
        with band("proj"):
            nc.sync.dma_start(khl_sb[HD:P, :, ts], khl_sb[0:HD, :, ts])
        # V transpose via PE identity matmul: [64, 128] -> [128, 64]
        if PROJ_PSUM == "w" and b == 0:
            tr_ps = pp.tile([P, 4, HD], BF16, tag="W", bufs=W_BUFS, name="trw")
        else:
            tr_ps = pp.tile([P, 4, HD], BF16, tag="PJ", bufs=PJ_BUFS, name="tr")
        for i in range(4):
            nc.tensor.transpose(
                tr_ps[:, i], vt_sb[:, i * P : (i + 1) * P], ident[:]
            )
        with band("proj"):
            nc.vector.tensor_copy(v1_sb[:, 4 * tb : 4 * tb + 4, 0:HD], tr_ps[:])

    def attn(st, qb):
        # ------------- attention scores + exp for q-block qb -----------
        b = st["b"]
        qt8_sb, khl_sb = st["qt8"], st["khl"]
        nkt = 4 * (qb + 1)  # causal: kpos tiles 0..nkt-1
        p4 = p_pool.tile([P, 2, 2, nkt, TB], BF16, tag=f"P{qb % 2}", bufs=1)
        with band("attn"):
            for kt in range(nkt):
                for pair in range(2):
                    p_sb = p4[:, pair]
                    ks = slice(kt * P, (kt + 1) * P)
                    dk = kt - qb * 4
                    off = max(dk, 0) * P  # first potentially-valid column
                    s_ps = pp.tile([P, 2, TB], F32, tag="S", bufs=S_BUFS)
                    qs = slice(qb * TB + off, (qb + 1) * TB)
                    # one fp8 DR matmul per head: (kh,kl) stationary x
                    # (qh,qh) moving = k . qh, K exact; psum = 8*score.
                    # The moving (qh,qh) pair is a stride-0 broadcast AP.
                    for half in range(2):
                        hp = slice(HD * half, HD * (half + 1))
                        qmv = qt8_sb[hp, pair, qs].unsqueeze(1)
                        nc.tensor.matmul(
                            s_ps[:, half, off:],
                            khl_sb[hp, :, ks],
                            qmv.broadcast_to([HD, 2, qmv.shape[2]]),
                            start=True,
                            stop=True,
                            perf_mode=DR,
                        )
                    nc.scalar.activation(
                        p_sb[:, :, kt, off:], s_ps[:, :, off:], AF.Exp,
                        scale=SEXP,
                    )
                    if dk >= 0:  # diagonal block: causal triangle mask
                        (nc.gpsimd if MASK_POOL else nc.vector).tensor_mul(
                            p_sb[:, :, kt, off : off + P],
                            p_sb[:, :, kt, off : off + P],
                            tri[:],
                        )
        return p4

    def attn_b(st, qb, p4):
        b = st["b"]
        v1_sb = st["v1"]
        ot_sb = ot_pool.tile([P, 2, TB], BF16, tag="ot")
        ot8h = ot8_pool.tile([P, 2, TB], FP8, tag="oh")
        ot8l = ot8_pool.tile([P, 2, TB], FP8, tag="ol")
        # --- phase B: PV accumulation, normalize, transpose ---
        def bj(j):
            for pair in range(2):
                p_sb = p4[:, pair]
                ktn = qb * 4 + j + 1  # kpos tiles 0..ktn-1
                o_n = on_pool.tile([P, 2, HD], BF16, tag="on")
                for half in range(2):
                    o_ps = pp.tile([P, HD + 1], F32, tag="O", bufs=O_BUFS)
                    with band("pv") if PV_BAND else _null():
                        for kt in range(ktn):
                            nc.tensor.matmul(
                                o_ps[:],
                                p_sb[:, half, kt, j * P : (j + 1) * P],
                                v1_sb[:, kt, :],
                                start=(kt == 0),
                                stop=(kt == ktn - 1),
                            )
                    rec = rc_pool.tile([P, 1], F32, tag="rec")
                    nc.vector.reciprocal(rec[:], o_ps[:, HD : HD + 1])
                    nc.vector.tensor_scalar(
                        o_n[:, half, :], o_ps[:, 0:HD], rec[:], 8.0,
                        mybir.AluOpType.mult, mybir.AluOpType.mult,
                    )
                js = slice(j * P, (j + 1) * P)
                if b == B - 1 and qb == 3 and j == 3:
                    # drain path: PE transpose instead of the xbar DMA
                    # transpose (~1.7 us latency); S is idle by then
                    tp = pp.tile(
                        [P, 2, TB], BF16, tag="S", bufs=S_BUFS,
                        name=f"tp{pair}",
                    )
                    nc.tensor.transpose(tp[:, 0, 0:P], o_n[:], ident128[:])
                    nc.vector.tensor_copy(ot_sb[:, pair, js], tp[:, 0, 0:P])
                else:
                    nc.sync.dma_start_transpose(ot_sb[:, pair, js], o_n[:])
                # SBUF-only fp8 split of ot (the piece of DVE work that
                # CAN move to Pool; GPSIMD cannot touch PSUM). The final
                # drain block stays on DVE for latency.
                drainb = DRAIN_OT8_DVE and b == B - 1 and qb == 3
                e8 = nc.gpsimd if (OT8_POOL and not drainb) else nc.vector
                e8.tensor_copy(ot8h[:, pair, js], ot_sb[:, pair, js])
                e8.tensor_sub(
                    ot8l[:, pair, js], ot_sb[:, pair, js], ot8h[:, pair, js]
                )
        return bj, (ot8h, ot8l)

    def _wo_j(st, qb, ots, j, split_stores=False):
        b = st["b"]
        ot8h, ot8l = ots
        rows = slice(qb * TB + j * P, qb * TB + (j + 1) * P)
        stg = stg_pool.tile([P, D], BF16, tag="stg")
        for nb in range(4):
            if b == 1 and qb == 2 and nb in (1, 3):
                wo_ps = pp.tile(
                    [P, TB], F32, tag="PJ", bufs=PJ_BUFS,
                    name=f"w2P{j}{nb}",
                )
            elif split_stores:
                # final block: every other psum tag is idle by now --
                # rotate through them so Wo is not W-recycle-bound
                if nb == 3 or nb == 1:
                    wo_ps = pp.tile(
                        [P, 2, TB], F32, tag="S", bufs=S_BUFS,
                        name=f"wS{j}{nb}",
                    )[:, 0]
                elif nb == 2:
                    wo_ps = pp.tile(
                        [P, TB], F32, tag="PJ", bufs=PJ_BUFS,
                        name=f"wP{j}",
                    )
                else:
                    wo_ps = pp.tile([P, TB], F32, tag="W", bufs=W_BUFS)
            else:
                wo_ps = pp.tile([P, TB], F32, tag="W", bufs=W_BUFS)
            ns = slice(nb * TB, (nb + 1) * TB)
            for i, (osb, wsb) in enumerate(
                ((ot8h, woh_sb), (ot8l, woh_sb), (ot8h, wol_sb))
            ):
                nc.tensor.matmul(
                    wo_ps[:],
                    osb[:, :, j * P : (j + 1) * P],
                    wsb[:, :, ns],
                    start=(i == 0),
                    stop=(i == 2),
                    perf_mode=DR,
                )
            if split_stores and (nb % 2 == 0 or not DRAIN_SPLIT):
                # final block: ACT is mostly idle (no exps left)
                nc.scalar.mul(stg[:, ns], wo_ps[:], 1.0 / (8.0 * WSCALE))
            else:
                nc.vector.tensor_scalar_mul(
                    stg[:, ns], wo_ps[:], 1.0 / (8.0 * WSCALE)
                )
            if split_stores:
                nc.sync.dma_start(
                    out[b, rows, nb * TB : (nb + 1) * TB],
                    stg[:, nb * TB : (nb + 1) * TB],
                )
        if not split_stores:
            nc.sync.dma_start(out[b, rows, :], stg[:])

    def ab_wo(st, qb, p4, split_stores=False, defer_wo_js=()):
        # Phase B + Wo, j-major so each row-block's Wo unlocks early.
        # defer_wo_js postpones those row-blocks' Wo to the returned thunk
        # (emitted later = lower priority = fills the exp-bound tail).
        bj, ots = attn_b(st, qb, p4)
        deferred = []
        for j in range(NT):
            bj(j)
            if j in defer_wo_js:
                deferred.append(j)
            else:
                _wo_j(st, qb, ots, j, split_stores)

        def run_deferred():
            for j in deferred:
                _wo_j(st, qb, ots, j, split_stores)

        return run_deferred

    # ---- global schedule. Emission order = scheduler priority. Rules:
    # * each proj is emitted BEFORE the neighboring ab_wo so its psum->SBUF
    #   copies (which gate the next attention block's scores) outrank the
    #   ab_wo normalize/Wo copies on the shared DVE queue;
    # * attn(qb+1) right before/after ab_wo(qb) as in the exp-stream
    #   pipeline; a P-tag is only reused after its readers are emitted;
    # * b1 projections fill b0's ACT-bound qb3 window; late Wo row-blocks
    #   are deferred into the underfilled b1-qb2/qb3 windows (each deferral
    #   lands before the ot8 buffer (bufs=2) it reads is recycled). ----
    s0 = batch_state(0)
    s1 = batch_state(1)
    if SCHED == "v5":
        proj(s0, 0)
        proj(s0, 1)
        p1 = attn(s0, 0)
        proj(s0, 2)
        p2 = attn(s0, 1)
        ab_wo(s0, 0, p1)
        proj(s0, 3)
        p3 = attn(s0, 2)
        ab_wo(s0, 1, p2)
        proj(s1, 0)
        p4_ = attn(s0, 3)
        ab_wo(s0, 2, p3)
        proj(s1, 1)
        d03 = ab_wo(s0, 3, p4_, defer_wo_js=(1, 2, 3))
        proj(s1, 2)
        proj(s1, 3)
        q1 = attn(s1, 0)
        q2 = attn(s1, 1)
        d10 = ab_wo(s1, 0, q1, defer_wo_js=(2, 3))
        d03()
        q3 = attn(s1, 2)
        d11 = ab_wo(s1, 1, q2, defer_wo_js=(2, 3))
        d10()
        q4_ = attn(s1, 3)
        d12 = ab_wo(s1, 2, q3, defer_wo_js=(2, 3))
        d11()
        d12()
        ab_wo(s1, 3, q4_, split_stores=True)
    elif SCHED == "v4":
        proj(s0, 0)
        proj(s0, 1)
        p1 = attn(s0, 0)
        proj(s0, 2)
        p2 = attn(s0, 1)
        ab_wo(s0, 0, p1)
        proj(s0, 3)
        p3 = attn(s0, 2)
        proj(s1, 0)
        ab_wo(s0, 1, p2)
        proj(s1, 1)
        p4_ = attn(s0, 3)
        ab_wo(s0, 2, p3)
        proj(s1, 2)
        proj(s1, 3)
        ab_wo(s0, 3, p4_)
        q1 = attn(s1, 0)
        q2 = attn(s1, 1)
        ab_wo(s1, 0, q1)
        q3 = attn(s1, 2)
        ab_wo(s1, 1, q2)
        q4_ = attn(s1, 3)
        dfr = ab_wo(s1, 2, q3, defer_wo_js=(2, 3) if DEFER else ())
        dfr()
        ab_wo(s1, 3, q4_, split_stores=True)
    elif SCHED == "v1":
        proj(s0, 0)
        proj(s0, 1)
        p1 = attn(s0, 0)
        proj(s0, 2)
        p2 = attn(s0, 1)
        ab_wo(s0, 0, p1)
        proj(s0, 3)
        p3 = attn(s0, 2)
        ab_wo(s0, 1, p2)
        proj(s1, 0)
        p4_ = attn(s0, 3)
        ab_wo(s0, 2, p3)
        proj(s1, 1)
        ab_wo(s0, 3, p4_)
        proj(s1, 2)
        proj(s1, 3)
        q1 = attn(s1, 0)
        q2 = attn(s1, 1)
        ab_wo(s1, 0, q1)
        q3 = attn(s1, 2)
        ab_wo(s1, 1, q2)
        q4_ = attn(s1, 3)
        dfr = ab_wo(s1, 2, q3, defer_wo_js=(2, 3) if DEFER else ())
        dfr()
        ab_wo(s1, 3, q4_, split_stores=True)
    else:  # v3
        proj(s0, 0)
        p1 = attn(s0, 0)
        proj(s0, 1)
        p2 = attn(s0, 1)
        proj(s0, 2)
        ab_wo(s0, 0, p1)
        p3 = attn(s0, 2)
        proj(s0, 3)
        ab_wo(s0, 1, p2)
        proj(s1, 0)
        p4_ = attn(s0, 3)
        proj(s1, 1)
        ab_wo(s0, 2, p3)
        proj(s1, 2)
        q1 = attn(s1, 0)
        d03 = ab_wo(s0, 3, p4_, defer_wo_js=(2, 3))
        q2 = attn(s1, 1)
        proj(s1, 3)
        d10 = ab_wo(s1, 0, q1, defer_wo_js=(2, 3))
        d03()
        q3 = attn(s1, 2)
        d11 = ab_wo(s1, 1, q2, defer_wo_js=(1, 2, 3))
        d10()
        q4_ = attn(s1, 3)
        d12 = ab_wo(s1, 2, q3, defer_wo_js=(1, 2, 3))
        d11()
        ab_wo(s1, 3, q4_, split_stores=True)
        d12()


_NC_CACHE = {}


def get_nc():
    if "nc" not in _NC_CACHE:
        nc = bacc.Bacc("TRN2", target_bir_lowering=False, debug=False)
        with tile.TileContext(nc) as tc, ExitStack() as ctx:
            build_kernel(ctx, tc)
        nc.compile()
        _NC_CACHE["nc"] = nc
    return _NC_CACHE["nc"]


def make_in_maps(x, Wq, Wk, Wv, Wo):
    FP8NP = ml_dtypes.float8_e4m3

    def fp8_split(a):
        hi = a.astype(FP8NP)
        lo = (a - hi.astype(np.float32)).astype(FP8NP)
        return hi, lo

    xT = np.ascontiguousarray(np.transpose(np.asarray(x, np.float32), (0, 2, 1)))
    xh, xl = fp8_split(xT)
    Wq, Wk, Wv, Wo = (np.asarray(w, np.float32) for w in (Wq, Wk, Wv, Wo))
    in_maps = []
    for g in range(NCORES):
        in_maps.append(
            {
                "xh": xh,
                "xl": xl,
                **dict(
                    zip(
                        ("wqh", "wql"),
                        fp8_split(
                            WSCALE * np.ascontiguousarray(Wq[:, g * DQ : (g + 1) * DQ])
                        ),
                    )
                ),
                **dict(
                    zip(
                        ("wkvh", "wkvl"),
                        fp8_split(
                            np.ascontiguousarray(
                                np.concatenate(
                                    [
                                        WSCALE_K * Wk[:, g * HD : (g + 1) * HD],
                                        WSCALE * Wv[:, g * HD : (g + 1) * HD],
                                    ],
                                    axis=1,
                                )
                            )
                        ),
                    )
                ),
                **dict(
                    zip(
                        ("woh", "wol"),
                        fp8_split(
                            WSCALE
                            * np.ascontiguousarray(Wo[g * DQ : (g + 1) * DQ, :])
                        ),
                    )
                ),
            }
        )
    return in_maps


def run(x, Wq, Wk, Wv, Wo, trace=False):
    nc = get_nc()
    in_maps = make_in_maps(x, Wq, Wk, Wv, Wo)
    res = run_bass_kernel_spmd(nc, in_maps, list(range(NCORES)), trace=trace)
    acc = np.zeros((B, T, D), np.float32)
    for r in res.results:
        acc += np.asarray(r["out"], dtype=np.float32)
    return acc, res


def kernel(x, Wq, Wk, Wv, Wo):
    return run(x, Wq, Wk, Wv, Wo)[0]


# revision 4
# speedup vs baseline: 1.0449x; 1.0415x over previous
"""GQA attention kernel for Trainium2, tensor-parallel across 8 NeuronCores.

Problem: B=2, T=2048, D=2048, H=32 q-heads, G=8 kv-heads (GQA, rep=4), hd=64,
causal softmax attention + output projection, fp32 I/O.

Sharding (one KV group per core):
  core g: Wq[:, g*256:(g+1)*256], Wk/Wv[:, g*64:(g+1)*64], Wo[g*256:(g+1)*256, :]
  Each core computes its 4 heads' attention + partial output projection;
  host sums the 8 partial outputs (row-parallel Wo => partial-sum unshard).

Changes vs the previous 223us version (-> 209us, rel err ~1.0e-2):
  * Scores in ONE fp8 DoubleRow matmul per (kt, head) instead of one bf16
    matmul (halves score PE cost): stationary = [kh; kl] (error-split K,
    exact), moving = (qh, qh) via a stride-0 broadcast AP (single fp8 Q at
    scale 1/2, ~2.4% quant noise -> ~1.0e-2 final rel err vs the 2e-2
    gate). K weights are host-scaled by 16 (not 64) so fp8(16k), |max|~80,
    stays inside e4m3's 448 range when split; the 1/64 unscale is folded
    into the exp's activation scale. Q/V/Wo keep WSCALE=64 and full
    3-pass accuracy.
  * Global software pipeline across batches: b=1's projections are emitted
    inside b=0's ACT-bound attention windows. Scheduler priority bands
    (tc.cur_priority): score->exp stream on top, projection psum->SBUF
    copies + x loads in the middle (they gate the next attention block),
    PV/Wo/stores as fill. Keeps the serial exp stream on ACT (the
    attention-phase bottleneck, ~146us) continuously fed.
  * The SBUF-only ot8 fp8 split and the causal-mask multiply run on Pool
    (GPSIMD cannot touch PSUM, so only those could move off DVE).
  * x loaded with 8-buf rotation (2 t-blocks of lookahead) so b1's
    projection pipeline is not serialized behind b0's.

Per-core dataflow otherwise matches the previous version: fp8 error-split
DoubleRow projections (hi@hi + lo@hi + hi@lo over ko-pairs), V transposed
via PE identity matmuls to v1 [kpos, 16, hd|1] with a ones column, PV with
P-slices stationary (65-cycle matmuls), DVE reciprocal normalize, o_n
transposed by DMA xbar, Wo partial via 3-pass fp8 DoubleRow.
"""

import os
import sys

import numpy as np

for _p in ("/opt/trn_rl_repo", "/root/.axon_site/_ro/trn_rl_repo"):
    if os.path.isdir(_p) and _p not in sys.path:
        sys.path.insert(0, _p)

import ml_dtypes  # noqa: E402

import concourse.bass as bass  # noqa: E402
import concourse.mybir as mybir  # noqa: E402
import concourse.tile as tile  # noqa: E402
from concourse import bacc  # noqa: E402
from concourse.bass_utils import run_bass_kernel_spmd  # noqa: E402
from concourse.masks import make_identity  # noqa: E402
from contextlib import ExitStack  # noqa: E402

B, T, D = 2, 2048, 2048
G, REP, HD = 8, 4, 64
DQ = REP * HD  # 256 q-dims per core
NCORES = 8
P = 128
TB = 512  # q/t block size
KO = D // P  # 16 contraction subtiles for projections
KQ = 4  # ko tiles per x DMA load
NT = T // TB  # 4 t-blocks
NKT = T // P  # 16 kpos tiles
F32 = mybir.dt.float32
BF16 = mybir.dt.bfloat16
FP8 = mybir.dt.float8e4
DR = mybir.MatmulPerfMode.DoubleRow
WSCALE = 64.0  # host multiplies Wq/Wv/Wo by this before fp8 split
WSCALE_K = 16.0  # K columns: smaller so fp8(16k) stays in e4m3 range
SQ = 0.5  # qh = q * SQ in fp8
SEXP = 1.0 / (8.0 * SQ * WSCALE_K)  # exp scale: psum = 8*SQ*WSCALE_K*s... see attn
AF = mybir.ActivationFunctionType
PJ_BUFS = 1
S_BUFS = 2
O_BUFS = 2
W_BUFS = 1
XT_BUFS = 8
XT1_BUFS = 2  # unused
KQ2 = 4  # effective KQ (overrides KQ below for sweeps)
BANDS = True      # use priority bands for attn stream / proj copies
OT8_POOL = True  # put the ot8 fp8 split on Pool instead of DVE
SCHED = "v1"     # emission order variant
DEFER = True      # defer late Wo row-blocks into the exp-bound tail
DRAIN_SPLIT = True  # split final-drain psum copies between ACT and DVE
PROJ_PSUM = "w"
HEAD_PJ = True
DRAIN_OT8_DVE = False
OT8_BUFS = 2
STG_BUFS = 2
WARMUP = 0  # junk PE transposes at t=0 (0 disables)
PV_BAND = False  # give PV matmuls a band above generic fill
MASK_POOL = True  # causal mask multiply on Pool instead of DVE  # "pj": proj chains+tr self-contained on PJ; "w": borrow W


def build_kernel(ctx, tc):
    nc = tc.nc
    from contextlib import contextmanager

    # Priority bands (lower = earlier = higher scheduler priority):
    #   [0, 500k)    attention score->exp stream (the serial ACT bottleneck;
    #                scores must preempt fill work the moment psum frees)
    #   [500k, 1M)   projection psum->SBUF copies + dup DMAs + xt loads
    #                (they gate the NEXT attention block's scores)
    #   [1M, ...)    everything else (proj matmuls, PV, Wo, stores) = fill
    tc.cur_priority = 1_000_000
    _bands = {"attn": [0], "proj": [500_000], "pv": [800_000]}

    @contextmanager
    def _null():
        yield

    @contextmanager
    def band(name):
        if not BANDS:
            yield
            return
        sv = tc.cur_priority
        tc.cur_priority = _bands[name][0]
        try:
            yield
        finally:
            _bands[name][0] = tc.cur_priority
            tc.cur_priority = sv
    xh = nc.dram_tensor("xh", [B, D, T], FP8, kind="ExternalInput").ap()
    xl = nc.dram_tensor("xl", [B, D, T], FP8, kind="ExternalInput").ap()
    wqh = nc.dram_tensor("wqh", [D, DQ], FP8, kind="ExternalInput").ap()
    wql = nc.dram_tensor("wql", [D, DQ], FP8, kind="ExternalInput").ap()
    wkvh = nc.dram_tensor("wkvh", [D, 2 * HD], FP8, kind="ExternalInput").ap()
    wkvl = nc.dram_tensor("wkvl", [D, 2 * HD], FP8, kind="ExternalInput").ap()
    woh = nc.dram_tensor("woh", [DQ, D], FP8, kind="ExternalInput").ap()
    wol = nc.dram_tensor("wol", [DQ, D], FP8, kind="ExternalInput").ap()
    out = nc.dram_tensor("out", [B, T, D], BF16, kind="ExternalOutput").ap()

    wpool = ctx.enter_context(tc.tile_pool(name="w", bufs=1))
    qt_pool = ctx.enter_context(tc.tile_pool(name="qt", bufs=2))
    kkt_pool = ctx.enter_context(tc.tile_pool(name="kkt", bufs=2))
    vt_pool = ctx.enter_context(tc.tile_pool(name="vt", bufs=2))
    v_pool = ctx.enter_context(tc.tile_pool(name="v", bufs=2))
    xt_pool = ctx.enter_context(tc.tile_pool(name="xt", bufs=XT_BUFS))
    p_pool = ctx.enter_context(tc.tile_pool(name="p", bufs=2))
    on_pool = ctx.enter_context(tc.tile_pool(name="on", bufs=3))
    rc_pool = ctx.enter_context(tc.tile_pool(name="rc", bufs=3))
    ot_pool = ctx.enter_context(tc.tile_pool(name="ot", bufs=2))
    ot8_pool = ctx.enter_context(tc.tile_pool(name="ot8", bufs=OT8_BUFS))
    stg_pool = ctx.enter_context(tc.tile_pool(name="stg", bufs=STG_BUFS))
    pp = ctx.enter_context(tc.tile_pool(name="pp", bufs=2, space="PSUM"))

    # persistent weights (SP/HWDGE queue; Pool is reserved for xt loads).
    # wq/wkv split into ko-chunks so the first matmuls wait only on chunk 0.
    wqh_sb = wpool.tile([P, KO, DQ], FP8, tag="wqh")
    wql_sb = wpool.tile([P, KO, DQ], FP8, tag="wql")
    wkvh_sb = wpool.tile([P, KO, 2 * HD], FP8, tag="wkvh")
    wkvl_sb = wpool.tile([P, KO, 2 * HD], FP8, tag="wkvl")
    for sb, dr in ((wqh_sb, wqh), (wkvh_sb, wkvh), (wql_sb, wql), (wkvl_sb, wkvl)):
        r = dr.rearrange("(ko p) m -> p ko m", p=P)
        for c in range(0, KO, KQ):
            nc.sync.dma_start(sb[:, c : c + KQ, :], r[:, c : c + KQ, :])
    woh_sb = wpool.tile([P, DQ // P, D], FP8, tag="woh")
    nc.sync.dma_start(woh_sb[:], woh.rearrange("(ko p) m -> p ko m", p=P))
    wol_sb = wpool.tile([P, DQ // P, D], FP8, tag="wol")
    nc.sync.dma_start(wol_sb[:], wol.rearrange("(ko p) m -> p ko m", p=P))
    # upper-triangular causal mask (keep f >= p), two identical copies so one
    # tensor_tensor covers both head halves of a pair at once
    ident = wpool.tile([HD, HD], BF16, tag="ident")
    make_identity(nc, ident[:])
    ident128 = wpool.tile([P, P], BF16, tag="id128")
    make_identity(nc, ident128[:])
    tri = wpool.tile([P, 2, P], BF16, tag="tri")
    nc.gpsimd.memset(tri[:], 1.0)
    for h in range(2):
        nc.gpsimd.affine_select(
            out=tri[:, h, :],
            in_=tri[:, h, :],
            compare_op=mybir.AluOpType.is_ge,
            fill=0.0,
            base=0,
            channel_multiplier=-1,
            pattern=[[1, P]],
        )

    def batch_state(b):
        st = {"b": b}
        # qt8: fp8 Q at scale SQ; dims [part(2 heads x 64), pair, T]. The
        # score DR matmul reads it through a stride-0 broadcast AP, so no
        # physical duplicate is needed.
        st["qt8"] = qt_pool.tile([P, 2, T], FP8, tag="qt", name=f"qt8_{b}")
        # khl: fp8 split of 16k; parts 0:64 = (kh, kl), 64:128 = duplicate
        st["khl"] = kkt_pool.tile([P, 2, T], FP8, tag="khl", name=f"khl_{b}")

        st["v1"] = v_pool.tile([P, NKT, HD + 1], BF16, tag="v1", name=f"v1_{b}")
        nc.gpsimd.memset(st["v1"][:, :, HD : HD + 1], 1.0)
        return st

    def proj(st, tb):
        # ---------------- projections for t-block tb ----------------
        b = st["b"]
        qt8_sb, khl_sb, v1_sb = st["qt8"], st["khl"], st["v1"]
        vt_sb = vt_pool.tile([HD, TB], BF16, tag="vt", name="vt")
        ts = slice(tb * TB, (tb + 1) * TB)
        xhs, xls = [], []
        with band("proj"):
            for src_t, lst, tag in ((xh, xhs, "xh"), (xl, xls, "xl")):
                for kq in range(KO // KQ):
                    xt = xt_pool.tile(
                        [P, KQ, TB], FP8, tag=tag, name="xt", bufs=XT_BUFS,
                    )
                    nc.gpsimd.dma_start(
                        xt[:],
                        src_t[b, kq * KQ * P : (kq + 1) * KQ * P, ts].rearrange(
                            "(q p) t -> p q t", p=P
                        ),
                    )
                    lst.append(xt)
        # three sequential accumulation chains (Q pair0, Q pair1, KV), each
        # as 3 fp8 DoubleRow passes (hi@hi + lo@hi + hi@lo) over ko-pairs.
        for ci, (whsb, wlsb, lo) in (
            (0, (wqh_sb, wql_sb, 0)),
            (2, (wkvh_sb, wkvl_sb, 0)),
            (1, (wqh_sb, wql_sb, P)),
        ):
            if b == 0 and tb == 0 and ci == 2:
                c_ps = pp.tile([P, TB], F32, tag="O", bufs=O_BUFS)
            elif PROJ_PSUM == "w" and b == 0 and tb >= 1 and ci == 1:
                c_ps = pp.tile([P, TB], F32, tag="W", bufs=W_BUFS)
            elif b == 0 and tb == 0 and ci < (1 if HEAD_PJ else 2):
                # before any attention exists the score psum is idle:
                # borrow S slots so the first three chains overlap
                sbig = pp.tile(
                    [P, 2, TB], F32, tag="S", bufs=S_BUFS, name=f"sb{ci}"
                )
                c_ps = sbig[:, 0]
            else:
                c_ps = pp.tile([P, TB], F32, tag="PJ", bufs=PJ_BUFS)
            passes = ((whsb, xhs), (whsb, xls), (wlsb, xhs))
            n_mm = len(passes) * (KO // 2)
            i = 0
            for wsb, xlist in passes:
                for kp in range(KO // 2):
                    ko = 2 * kp
                    nc.tensor.matmul(
                        c_ps[:],
                        wsb[:, ko : ko + 2, lo : lo + P],
                        xlist[ko // KQ][:, ko % KQ : ko % KQ + 2, :],
                        start=(i == 0),
                        stop=(i == n_mm - 1),
                        perf_mode=DR,
                    )
                    i += 1
            with band("proj"):
                if ci < 2:
                    # qh = q * SQ in fp8 (psum holds 64q)
                    nc.vector.tensor_scalar_mul(
                        qt8_sb[:, ci, ts], c_ps[:], SQ / WSCALE
                    )
                else:
                    # K psum holds 16k (host scaled Wk by 16): split to fp8
                    nc.vector.tensor_copy(khl_sb[0:HD, 0, ts], c_ps[0:HD, :])
                    nc.vector.tensor_tensor(
                        out=khl_sb[0:HD, 1, ts],
                        in0=c_ps[0:HD, :],
                        in1=khl_sb[0:HD, 0, ts],
                        op=mybir.AluOpType.subtract,
                    )
                    nc.vector.tensor_scalar_mul(
                        vt_sb[:], c_ps[HD:P, :], 1.0 / WSCALE
                    )
        # duplicate khl to partitions 64..127 (SBUF->SBUF DMA) so each
        # head-half's DR matmul has its stationary on its own partitions
        with band("proj"):
            nc.sync.dma_start(khl_sb[HD:P, :, ts], khl_sb[0:HD, :, ts])
        # V transpose via PE identity matmul: [64, 128] -> [128, 64]
        if PROJ_PSUM == "w" and b == 0:
            tr_ps = pp.tile([P, 4, HD], BF16, tag="W", bufs=W_BUFS, name="trw")
        else:
            tr_ps = pp.tile([P, 4, HD], BF16, tag="PJ", bufs=PJ_BUFS, name="tr")
        for i in range(4):
            nc.tensor.transpose(
                tr_ps[:, i], vt_sb[:, i * P : (i + 1) * P], ident[:]
            )
        with band("proj"):
            nc.vector.tensor_copy(v1_sb[:, 4 * tb : 4 * tb + 4, 0:HD], tr_ps[:])

    def attn(st, qb):
        # ------------- attention scores + exp for q-block qb -----------
        b = st["b"]
        qt8_sb, khl_sb = st["qt8"], st["khl"]
        nkt = 4 * (qb + 1)  # causal: kpos tiles 0..nkt-1
        p4 = p_pool.tile([P, 2, 2, nkt, TB], BF16, tag=f"P{qb % 2}", bufs=1)
        with band("attn"):
            for kt in range(nkt):
                for pair in range(2):
                    p_sb = p4[:, pair]
                    ks = slice(kt * P, (kt + 1) * P)
                    dk = kt - qb * 4
                    off = max(dk, 0) * P  # first potentially-valid column
                    s_ps = pp.tile([P, 2, TB], F32, tag="S", bufs=S_BUFS)
                    qs = slice(qb * TB + off, (qb + 1) * TB)
                    # one fp8 DR matmul per head: (kh,kl) stationary x
                    # (qh,qh) moving = k . qh, K exact; psum = 8*score.
                    # The moving (qh,qh) pair is a stride-0 broadcast AP.
                    for half in range(2):
                        hp = slice(HD * half, HD * (half + 1))
                        qmv = qt8_sb[hp, pair, qs].unsqueeze(1)
                        nc.tensor.matmul(
                            s_ps[:, half, off:],
                            khl_sb[hp, :, ks],
                            qmv.broadcast_to([HD, 2, qmv.shape[2]]),
                            start=True,
                            stop=True,
                            perf_mode=DR,
                        )
                    nc.scalar.activation(
                        p_sb[:, :, kt, off:], s_ps[:, :, off:], AF.Exp,
                        scale=SEXP,
                    )
                    if dk >= 0:  # diagonal block: causal triangle mask
                        (nc.gpsimd if MASK_POOL else nc.vector).tensor_mul(
                            p_sb[:, :, kt, off : off + P],
                            p_sb[:, :, kt, off : off + P],
                            tri[:],
                        )
        return p4

    def attn_b(st, qb, p4):
        b = st["b"]
        v1_sb = st["v1"]
        ot_sb = ot_pool.tile([P, 2, TB], BF16, tag="ot")
        ot8h = ot8_pool.tile([P, 2, TB], FP8, tag="oh")
        ot8l = ot8_pool.tile([P, 2, TB], FP8, tag="ol")
        # --- phase B: PV accumulation, normalize, transpose ---
        def bj(j):
            for pair in range(2):
                p_sb = p4[:, pair]
                ktn = qb * 4 + j + 1  # kpos tiles 0..ktn-1
                o_n = on_pool.tile([P, 2, HD], BF16, tag="on")
                for half in range(2):
                    o_ps = pp.tile([P, HD + 1], F32, tag="O", bufs=O_BUFS)
                    with band("pv") if PV_BAND else _null():
                        for kt in range(ktn):
                            nc.tensor.matmul(
                                o_ps[:],
                                p_sb[:, half, kt, j * P : (j + 1) * P],
                                v1_sb[:, kt, :],
                                start=(kt == 0),
                                stop=(kt == ktn - 1),
                            )
                    rec = rc_pool.tile([P, 1], F32, tag="rec")
                    nc.vector.reciprocal(rec[:], o_ps[:, HD : HD + 1])
                    nc.vector.tensor_scalar(
                        o_n[:, half, :], o_ps[:, 0:HD], rec[:], 8.0,
                        mybir.AluOpType.mult, mybir.AluOpType.mult,
                    )
                js = slice(j * P, (j + 1) * P)
                if b == B - 1 and qb == 3 and j == 3:
                    # drain path: PE transpose instead of the xbar DMA
                    # transpose (~1.7 us latency); S is idle by then
                    tp = pp.tile(
                        [P, 2, TB], BF16, tag="S", bufs=S_BUFS,
                        name=f"tp{pair}",
                    )
                    nc.tensor.transpose(tp[:, 0, 0:P], o_n[:], ident128[:])
                    nc.vector.tensor_copy(ot_sb[:, pair, js], tp[:, 0, 0:P])
                else:
                    nc.sync.dma_start_transpose(ot_sb[:, pair, js], o_n[:])
                # SBUF-only fp8 split of ot (the piece of DVE work that
                # CAN move to Pool; GPSIMD cannot touch PSUM). The final
                # drain block stays on DVE for latency.
                drainb = DRAIN_OT8_DVE and b == B - 1 and qb == 3
                e8 = nc.gpsimd if (OT8_POOL and not drainb) else nc.vector
                e8.tensor_copy(ot8h[:, pair, js], ot_sb[:, pair, js])
                e8.tensor_sub(
                    ot8l[:, pair, js], ot_sb[:, pair, js], ot8h[:, pair, js]
                )
        return bj, (ot8h, ot8l)

    def _wo_j(st, qb, ots, j, split_stores=False):
        b = st["b"]
        ot8h, ot8l = ots
        rows = slice(qb * TB + j * P, qb * TB + (j + 1) * P)
        stg = stg_pool.tile([P, D], BF16, tag="stg")
        for nb in range(4):
            if b == 1 and qb == 2 and nb in (1, 3):
                wo_ps = pp.tile(
                    [P, TB], F32, tag="PJ", bufs=PJ_BUFS,
                    name=f"w2P{j}{nb}",
                )
            elif split_stores:
                # final block: every other psum tag is idle by now --
                # rotate through them so Wo is not W-recycle-bound
                if nb == 3 or nb == 1:
                    wo_ps = pp.tile(
                        [P, 2, TB], F32, tag="S", bufs=S_BUFS,
                        name=f"wS{j}{nb}",
                    )[:, 0]
                elif nb == 2:
                    wo_ps = pp.tile(
                        [P, TB], F32, tag="PJ", bufs=PJ_BUFS,
                        name=f"wP{j}",
                    )
                else:
                    wo_ps = pp.tile([P, TB], F32, tag="W", bufs=W_BUFS)
            else:
                wo_ps = pp.tile([P, TB], F32, tag="W", bufs=W_BUFS)
            ns = slice(nb * TB, (nb + 1) * TB)
            for i, (osb, wsb) in enumerate(
                ((ot8h, woh_sb), (ot8l, woh_sb), (ot8h, wol_sb))
            ):
                nc.tensor.matmul(
                    wo_ps[:],
                    osb[:, :, j * P : (j + 1) * P],
                    wsb[:, :, ns],
                    start=(i == 0),
                    stop=(i == 2),
                    perf_mode=DR,
                )
            if split_stores and (nb % 2 == 0 or not DRAIN_SPLIT):
                # final block: ACT is mostly idle (no exps left)
                nc.scalar.mul(stg[:, ns], wo_ps[:], 1.0 / (8.0 * WSCALE))
            else:
                nc.vector.tensor_scalar_mul(
                    stg[:, ns], wo_ps[:], 1.0 / (8.0 * WSCALE)
                )
            if split_stores:
                nc.sync.dma_start(
                    out[b, rows, nb * TB : (nb + 1) * TB],
                    stg[:, nb * TB : (nb + 1) * TB],
                )
        if not split_stores:
            nc.sync.dma_start(out[b, rows, :], stg[:])

    def ab_wo(st, qb, p4, split_stores=False, defer_wo_js=()):
        # Phase B + Wo, j-major so each row-block's Wo unlocks early.
        # defer_wo_js postpones those row-blocks' Wo to the returned thunk
        # (emitted later = lower priority = fills the exp-bound tail).
        bj, ots = attn_b(st, qb, p4)
        deferred = []
        for j in range(NT):
            bj(j)
            if j in defer_wo_js:
                deferred.append(j)
            else:
                _wo_j(st, qb, ots, j, split_stores)

        def run_deferred():
            for j in deferred:
                _wo_j(st, qb, ots, j, split_stores)

        return run_deferred

    # ---- global schedule. Emission order = scheduler priority. Rules:
    # * each proj is emitted BEFORE the neighboring ab_wo so its psum->SBUF
    #   copies (which gate the next attention block's scores) outrank the
    #   ab_wo normalize/Wo copies on the shared DVE queue;
    # * attn(qb+1) right before/after ab_wo(qb) as in the exp-stream
    #   pipeline; a P-tag is only reused after its readers are emitted;
    # * b1 projections fill b0's ACT-bound qb3 window; late Wo row-blocks
    #   are deferred into the underfilled b1-qb2/qb3 windows (each deferral
    #   lands before the ot8 buffer (bufs=2) it reads is recycled). ----
    s0 = batch_state(0)
    s1 = batch_state(1)
    if SCHED == "v5":
        proj(s0, 0)
        proj(s0, 1)
        p1 = attn(s0, 0)
        proj(s0, 2)
        p2 = attn(s0, 1)
        ab_wo(s0, 0, p1)
        proj(s0, 3)
        p3 = attn(s0, 2)
        ab_wo(s0, 1, p2)
        proj(s1, 0)
        p4_ = attn(s0, 3)
        ab_wo(s0, 2, p3)
        proj(s1, 1)
        d03 = ab_wo(s0, 3, p4_, defer_wo_js=(1, 2, 3))
        proj(s1, 2)
        proj(s1, 3)
        q1 = attn(s1, 0)
        q2 = attn(s1, 1)
        d10 = ab_wo(s1, 0, q1, defer_wo_js=(2, 3))
        d03()
        q3 = attn(s1, 2)
        d11 = ab_wo(s1, 1, q2, defer_wo_js=(2, 3))
        d10()
        q4_ = attn(s1, 3)
        d12 = ab_wo(s1, 2, q3, defer_wo_js=(2, 3))
        d11()
        d12()
        ab_wo(s1, 3, q4_, split_stores=True)
    elif SCHED == "v4":
        proj(s0, 0)
        proj(s0, 1)
        p1 = attn(s0, 0)
        proj(s0, 2)
        p2 = attn(s0, 1)
        ab_wo(s0, 0, p1)
        proj(s0, 3)
        p3 = attn(s0, 2)
        proj(s1, 0)
        ab_wo(s0, 1, p2)
        proj(s1, 1)
        p4_ = attn(s0, 3)
        ab_wo(s0, 2, p3)
        proj(s1, 2)
        proj(s1, 3)
        ab_wo(s0, 3, p4_)
        q1 = attn(s1, 0)
        q2 = attn(s1, 1)
        ab_wo(s1, 0, q1)
        q3 = attn(s1, 2)
        ab_wo(s1, 1, q2)
        q4_ = attn(s1, 3)
        dfr = ab_wo(s1, 2, q3, defer_wo_js=(2, 3) if DEFER else ())
        dfr()
        ab_wo(s1, 3, q4_, split_stores=True)
    elif SCHED == "v1":
        proj(s0, 0)
        proj(s0, 1)
        p1 = attn(s0, 0)
        proj(s0, 2)
        p2 = attn(s0, 1)
        ab_wo(s0, 0, p1)
        proj(s0, 3)
        p3 = attn(s0, 2)
        ab_wo(s0, 1, p2)
        proj(s1, 0)
        p4_ = attn(s0, 3)
        ab_wo(s0, 2, p3)
        proj(s1, 1)
        ab_wo(s0, 3, p4_)
        proj(s1, 2)
        proj(s1, 3)
        q1 = attn(s1, 0)
        q2 = attn(s1, 1)
        ab_wo(s1, 0, q1)
        q3 = attn(s1, 2)
        ab_wo(s1, 1, q2)
        q4_ = attn(s1, 3)
        dfr = ab_wo(s1, 2, q3, defer_wo_js=(2, 3) if DEFER else ())
        dfr()
        ab_wo(s1, 3, q4_, split_stores=True)
    else:  # v3
        proj(s0, 0)
        p1 = attn(s0, 0)
        proj(s0, 1)
        p2 = attn(s0, 1)
        proj(s0, 2)
        ab_wo(s0, 0, p1)
        p3 = attn(s0, 2)
        proj(s0, 3)
        ab_wo(s0, 1, p2)
        proj(s1, 0)
        p4_ = attn(s0, 3)
        proj(s1, 1)
        ab_wo(s0, 2, p3)
        proj(s1, 2)
        q1 = attn(s1, 0)
        d03 = ab_wo(s0, 3, p4_, defer_wo_js=(2, 3))
        q2 = attn(s1, 1)
        proj(s1, 3)
        d10 = ab_wo(s1, 0, q1, defer_wo_js=(2, 3))
        d03()
        q3 = attn(s1, 2)
        d11 = ab_wo(s1, 1, q2, defer_wo_js=(1, 2, 3))
        d10()
        q4_ = attn(s1, 3)
        d12 = ab_wo(s1, 2, q3, defer_wo_js=(1, 2, 3))
        d11()
        ab_wo(s1, 3, q4_, split_stores=True)
        d12()


_NC_CACHE = {}


def get_nc():
    if "nc" not in _NC_CACHE:
        nc = bacc.Bacc("TRN2", target_bir_lowering=False, debug=False)
        with tile.TileContext(nc) as tc, ExitStack() as ctx:
            build_kernel(ctx, tc)
        nc.compile()
        _NC_CACHE["nc"] = nc
    return _NC_CACHE["nc"]


def make_in_maps(x, Wq, Wk, Wv, Wo):
    FP8NP = ml_dtypes.float8_e4m3

    def fp8_split(a):
        hi = a.astype(FP8NP)
        lo = (a - hi.astype(np.float32)).astype(FP8NP)
        return hi, lo

    xT = np.ascontiguousarray(np.transpose(np.asarray(x, np.float32), (0, 2, 1)))
    xh, xl = fp8_split(xT)
    Wq, Wk, Wv, Wo = (np.asarray(w, np.float32) for w in (Wq, Wk, Wv, Wo))
    in_maps = []
    for g in range(NCORES):
        in_maps.append(
            {
                "xh": xh,
                "xl": xl,
                **dict(
                    zip(
                        ("wqh", "wql"),
                        fp8_split(
                            WSCALE * np.ascontiguousarray(Wq[:, g * DQ : (g + 1) * DQ])
                        ),
                    )
                ),
                **dict(
                    zip(
                        ("wkvh", "wkvl"),
                        fp8_split(
                            np.ascontiguousarray(
                                np.concatenate(
                                    [
                                        WSCALE_K * Wk[:, g * HD : (g + 1) * HD],
                                        WSCALE * Wv[:, g * HD : (g + 1) * HD],
                                    ],
                                    axis=1,
                                )
                            )
                        ),
                    )
                ),
                **dict(
                    zip(
                        ("woh", "wol"),
                        fp8_split(
                            WSCALE
                            * np.ascontiguousarray(Wo[g * DQ : (g + 1) * DQ, :])
                        ),
                    )
                ),
            }
        )
    return in_maps


def run(x, Wq, Wk, Wv, Wo, trace=False):
    nc = get_nc()
    in_maps = make_in_maps(x, Wq, Wk, Wv, Wo)
    res = run_bass_kernel_spmd(nc, in_maps, list(range(NCORES)), trace=trace)
    acc = np.zeros((B, T, D), np.float32)
    for r in res.results:
        acc += np.asarray(r["out"], dtype=np.float32)
    return acc, res


def kernel(x, Wq, Wk, Wv, Wo):
    return run(x, Wq, Wk, Wv, Wo)[0]


# revision 5
# speedup vs baseline: 1.0544x; 1.0091x over previous
"""GQA attention kernel for Trainium2, tensor-parallel across 8 NeuronCores.

Problem: B=2, T=2048, D=2048, H=32 q-heads, G=8 kv-heads (GQA, rep=4), hd=64,
causal softmax attention + output projection, fp32 I/O.

Sharding (one KV group per core):
  core g: Wq[:, g*256:(g+1)*256], Wk/Wv[:, g*64:(g+1)*64], Wo[g*256:(g+1)*256, :]
  Each core computes its 4 heads' attention + partial output projection;
  host sums the 8 partial outputs (row-parallel Wo => partial-sum unshard).

Changes vs the previous 223us version (-> 200.7us, rel err 1.35e-2):
  * Scores in ONE fp8 DoubleRow matmul per (kt, head) instead of one bf16
    matmul (halves score PE cost): stationary = [kh; kl] (error-split K,
    exact), moving = (qh, qh) via a stride-0 broadcast AP (single fp8 Q at
    scale 1/2, ~2.4% quant noise -> ~1.0e-2 final rel err vs the 2e-2
    gate). K weights are host-scaled by 16 (not 64) so fp8(16k), |max|~80,
    stays inside e4m3's 448 range when split; the 1/64 unscale is folded
    into the exp's activation scale. V/Wo keep WSCALE=64 and full 3-pass
    accuracy; the Q projection runs 2-pass (drops the w-lo correction,
    whose error is below the score-level fp8 quantization floor anyway),
    shaving another 13.7us of PE time. Final rel err 1.35e-2 vs 2e-2 gate.
  * Global software pipeline across batches: b=1's projections are emitted
    inside b=0's ACT-bound attention windows. Scheduler priority bands
    (tc.cur_priority): score->exp stream on top, projection psum->SBUF
    copies + x loads in the middle (they gate the next attention block),
    PV/Wo/stores as fill. Keeps the serial exp stream on ACT (the
    attention-phase bottleneck, ~146us) continuously fed.
  * The SBUF-only ot8 fp8 split and the causal-mask multiply run on Pool
    (GPSIMD cannot touch PSUM, so only those could move off DVE).
  * x loaded with 8-buf rotation (2 t-blocks of lookahead) so b1's
    projection pipeline is not serialized behind b0's.

Per-core dataflow otherwise matches the previous version: fp8 error-split
DoubleRow projections (hi@hi + lo@hi + hi@lo over ko-pairs), V transposed
via PE identity matmuls to v1 [kpos, 16, hd|1] with a ones column, PV with
P-slices stationary (65-cycle matmuls), DVE reciprocal normalize, o_n
transposed by DMA xbar, Wo partial via 3-pass fp8 DoubleRow.
"""

import os
import sys

import numpy as np

for _p in ("/opt/trn_rl_repo", "/root/.axon_site/_ro/trn_rl_repo"):
    if os.path.isdir(_p) and _p not in sys.path:
        sys.path.insert(0, _p)

import ml_dtypes  # noqa: E402

import concourse.bass as bass  # noqa: E402
import concourse.mybir as mybir  # noqa: E402
import concourse.tile as tile  # noqa: E402
from concourse import bacc  # noqa: E402
from concourse.bass_utils import run_bass_kernel_spmd  # noqa: E402
from concourse.masks import make_identity  # noqa: E402
from contextlib import ExitStack  # noqa: E402

B, T, D = 2, 2048, 2048
G, REP, HD = 8, 4, 64
DQ = REP * HD  # 256 q-dims per core
NCORES = 8
P = 128
TB = 512  # q/t block size
KO = D // P  # 16 contraction subtiles for projections
KQ = 4  # ko tiles per x DMA load
NT = T // TB  # 4 t-blocks
NKT = T // P  # 16 kpos tiles
F32 = mybir.dt.float32
BF16 = mybir.dt.bfloat16
FP8 = mybir.dt.float8e4
DR = mybir.MatmulPerfMode.DoubleRow
WSCALE = 64.0  # host multiplies Wq/Wv/Wo by this before fp8 split
WSCALE_K = 16.0  # K columns: smaller so fp8(16k) stays in e4m3 range
SQ = 0.5  # qh = q * SQ in fp8
SEXP = 1.0 / (8.0 * SQ * WSCALE_K)  # exp scale: psum = 8*SQ*WSCALE_K*s... see attn
AF = mybir.ActivationFunctionType
PJ_BUFS = 1
S_BUFS = 2
O_BUFS = 2
W_BUFS = 1
XT_BUFS = 8
XT1_BUFS = 2  # unused
KQ2 = 4  # effective KQ (overrides KQ below for sweeps)
BANDS = True      # use priority bands for attn stream / proj copies
OT8_POOL = True  # put the ot8 fp8 split on Pool instead of DVE
SCHED = "v1"     # emission order variant
DEFER = True      # defer late Wo row-blocks into the exp-bound tail
DRAIN_SPLIT = True  # split final-drain psum copies between ACT and DVE
PROJ_PSUM = "w"
HEAD_PJ = True
DRAIN_OT8_DVE = False
OT8_BUFS = 2
STG_BUFS = 2
WARMUP = 0  # junk PE transposes at t=0 (0 disables)
PV_BAND = False  # give PV matmuls a band above generic fill
QP2 = True  # Q projection 2-pass (drop w-lo pass)
MASK_POOL = True  # causal mask multiply on Pool instead of DVE  # "pj": proj chains+tr self-contained on PJ; "w": borrow W


def build_kernel(ctx, tc):
    nc = tc.nc
    from contextlib import contextmanager

    # Priority bands (lower = earlier = higher scheduler priority):
    #   [0, 500k)    attention score->exp stream (the serial ACT bottleneck;
    #                scores must preempt fill work the moment psum frees)
    #   [500k, 1M)   projection psum->SBUF copies + dup DMAs + xt loads
    #                (they gate the NEXT attention block's scores)
    #   [1M, ...)    everything else (proj matmuls, PV, Wo, stores) = fill
    tc.cur_priority = 1_000_000
    _bands = {"attn": [0], "proj": [500_000], "pv": [800_000]}

    @contextmanager
    def _null():
        yield

    @contextmanager
    def band(name):
        if not BANDS:
            yield
            return
        sv = tc.cur_priority
        tc.cur_priority = _bands[name][0]
        try:
            yield
        finally:
            _bands[name][0] = tc.cur_priority
            tc.cur_priority = sv
    xh = nc.dram_tensor("xh", [B, D, T], FP8, kind="ExternalInput").ap()
    xl = nc.dram_tensor("xl", [B, D, T], FP8, kind="ExternalInput").ap()
    wqh = nc.dram_tensor("wqh", [D, DQ], FP8, kind="ExternalInput").ap()
    wql = nc.dram_tensor("wql", [D, DQ], FP8, kind="ExternalInput").ap()
    wkvh = nc.dram_tensor("wkvh", [D, 2 * HD], FP8, kind="ExternalInput").ap()
    wkvl = nc.dram_tensor("wkvl", [D, 2 * HD], FP8, kind="ExternalInput").ap()
    woh = nc.dram_tensor("woh", [DQ, D], FP8, kind="ExternalInput").ap()
    wol = nc.dram_tensor("wol", [DQ, D], FP8, kind="ExternalInput").ap()
    out = nc.dram_tensor("out", [B, T, D], BF16, kind="ExternalOutput").ap()

    wpool = ctx.enter_context(tc.tile_pool(name="w", bufs=1))
    qt_pool = ctx.enter_context(tc.tile_pool(name="qt", bufs=2))
    kkt_pool = ctx.enter_context(tc.tile_pool(name="kkt", bufs=2))
    vt_pool = ctx.enter_context(tc.tile_pool(name="vt", bufs=2))
    v_pool = ctx.enter_context(tc.tile_pool(name="v", bufs=2))
    xt_pool = ctx.enter_context(tc.tile_pool(name="xt", bufs=XT_BUFS))
    p_pool = ctx.enter_context(tc.tile_pool(name="p", bufs=2))
    on_pool = ctx.enter_context(tc.tile_pool(name="on", bufs=3))
    rc_pool = ctx.enter_context(tc.tile_pool(name="rc", bufs=3))
    ot_pool = ctx.enter_context(tc.tile_pool(name="ot", bufs=2))
    ot8_pool = ctx.enter_context(tc.tile_pool(name="ot8", bufs=OT8_BUFS))
    stg_pool = ctx.enter_context(tc.tile_pool(name="stg", bufs=STG_BUFS))
    pp = ctx.enter_context(tc.tile_pool(name="pp", bufs=2, space="PSUM"))

    # persistent weights (SP/HWDGE queue; Pool is reserved for xt loads).
    # wq/wkv split into ko-chunks so the first matmuls wait only on chunk 0.
    wqh_sb = wpool.tile([P, KO, DQ], FP8, tag="wqh")
    wql_sb = wpool.tile([P, KO, DQ], FP8, tag="wql")
    wkvh_sb = wpool.tile([P, KO, 2 * HD], FP8, tag="wkvh")
    wkvl_sb = wpool.tile([P, KO, 2 * HD], FP8, tag="wkvl")
    for sb, dr in ((wqh_sb, wqh), (wkvh_sb, wkvh), (wql_sb, wql), (wkvl_sb, wkvl)):
        r = dr.rearrange("(ko p) m -> p ko m", p=P)
        for c in range(0, KO, KQ):
            nc.sync.dma_start(sb[:, c : c + KQ, :], r[:, c : c + KQ, :])
    woh_sb = wpool.tile([P, DQ // P, D], FP8, tag="woh")
    nc.sync.dma_start(woh_sb[:], woh.rearrange("(ko p) m -> p ko m", p=P))
    wol_sb = wpool.tile([P, DQ // P, D], FP8, tag="wol")
    nc.sync.dma_start(wol_sb[:], wol.rearrange("(ko p) m -> p ko m", p=P))
    # upper-triangular causal mask (keep f >= p), two identical copies so one
    # tensor_tensor covers both head halves of a pair at once
    ident = wpool.tile([HD, HD], BF16, tag="ident")
    make_identity(nc, ident[:])
    ident128 = wpool.tile([P, P], BF16, tag="id128")
    make_identity(nc, ident128[:])
    tri = wpool.tile([P, 2, P], BF16, tag="tri")
    nc.gpsimd.memset(tri[:], 1.0)
    for h in range(2):
        nc.gpsimd.affine_select(
            out=tri[:, h, :],
            in_=tri[:, h, :],
            compare_op=mybir.AluOpType.is_ge,
            fill=0.0,
            base=0,
            channel_multiplier=-1,
            pattern=[[1, P]],
        )

    def batch_state(b):
        st = {"b": b}
        # qt8: fp8 Q at scale SQ; dims [part(2 heads x 64), pair, T]. The
        # score DR matmul reads it through a stride-0 broadcast AP, so no
        # physical duplicate is needed.
        st["qt8"] = qt_pool.tile([P, 2, T], FP8, tag="qt", name=f"qt8_{b}")
        # khl: fp8 split of 16k; parts 0:64 = (kh, kl), 64:128 = duplicate
        st["khl"] = kkt_pool.tile([P, 2, T], FP8, tag="khl", name=f"khl_{b}")

        st["v1"] = v_pool.tile([P, NKT, HD + 1], BF16, tag="v1", name=f"v1_{b}")
        nc.gpsimd.memset(st["v1"][:, :, HD : HD + 1], 1.0)
        return st

    def proj(st, tb):
        # ---------------- projections for t-block tb ----------------
        b = st["b"]
        qt8_sb, khl_sb, v1_sb = st["qt8"], st["khl"], st["v1"]
        vt_sb = vt_pool.tile([HD, TB], BF16, tag="vt", name="vt")
        ts = slice(tb * TB, (tb + 1) * TB)
        xhs, xls = [], []
        with band("proj"):
            for src_t, lst, tag in ((xh, xhs, "xh"), (xl, xls, "xl")):
                for kq in range(KO // KQ):
                    xt = xt_pool.tile(
                        [P, KQ, TB], FP8, tag=tag, name="xt", bufs=XT_BUFS,
                    )
                    nc.gpsimd.dma_start(
                        xt[:],
                        src_t[b, kq * KQ * P : (kq + 1) * KQ * P, ts].rearrange(
                            "(q p) t -> p q t", p=P
                        ),
                    )
                    lst.append(xt)
        # three sequential accumulation chains (Q pair0, Q pair1, KV), each
        # as 3 fp8 DoubleRow passes (hi@hi + lo@hi + hi@lo) over ko-pairs.
        for ci, (whsb, wlsb, lo) in (
            (0, (wqh_sb, wql_sb, 0)),
            (2, (wkvh_sb, wkvl_sb, 0)),
            (1, (wqh_sb, wql_sb, P)),
        ):
            if b == 0 and tb == 0 and ci == 2:
                c_ps = pp.tile([P, TB], F32, tag="O", bufs=O_BUFS)
            elif PROJ_PSUM == "w" and b == 0 and tb >= 1 and ci == 1:
                c_ps = pp.tile([P, TB], F32, tag="W", bufs=W_BUFS)
            elif b == 0 and tb == 0 and ci < (1 if HEAD_PJ else 2):
                # before any attention exists the score psum is idle:
                # borrow S slots so the first three chains overlap
                sbig = pp.tile(
                    [P, 2, TB], F32, tag="S", bufs=S_BUFS, name=f"sb{ci}"
                )
                c_ps = sbig[:, 0]
            else:
                c_ps = pp.tile([P, TB], F32, tag="PJ", bufs=PJ_BUFS)
            if QP2 and ci < 2:
                # Q tolerates 2-pass (its fp8 score quantization already
                # dominates); drops the w-lo correction pass
                passes = ((whsb, xhs), (whsb, xls))
            else:
                passes = ((whsb, xhs), (whsb, xls), (wlsb, xhs))
            n_mm = len(passes) * (KO // 2)
            i = 0
            for wsb, xlist in passes:
                for kp in range(KO // 2):
                    ko = 2 * kp
                    nc.tensor.matmul(
                        c_ps[:],
                        wsb[:, ko : ko + 2, lo : lo + P],
                        xlist[ko // KQ][:, ko % KQ : ko % KQ + 2, :],
                        start=(i == 0),
                        stop=(i == n_mm - 1),
                        perf_mode=DR,
                    )
                    i += 1
            with band("proj"):
                if ci < 2:
                    # qh = q * SQ in fp8 (psum holds 64q)
                    nc.vector.tensor_scalar_mul(
                        qt8_sb[:, ci, ts], c_ps[:], SQ / WSCALE
                    )
                else:
                    # K psum holds 16k (host scaled Wk by 16): split to fp8
                    nc.vector.tensor_copy(khl_sb[0:HD, 0, ts], c_ps[0:HD, :])
                    nc.vector.tensor_tensor(
                        out=khl_sb[0:HD, 1, ts],
                        in0=c_ps[0:HD, :],
                        in1=khl_sb[0:HD, 0, ts],
                        op=mybir.AluOpType.subtract,
                    )
                    nc.vector.tensor_scalar_mul(
                        vt_sb[:], c_ps[HD:P, :], 1.0 / WSCALE
                    )
        # duplicate khl to partitions 64..127 (SBUF->SBUF DMA) so each
        # head-half's DR matmul has its stationary on its own partitions
        with band("proj"):
            nc.sync.dma_start(khl_sb[HD:P, :, ts], khl_sb[0:HD, :, ts])
        # V transpose via PE identity matmul: [64, 128] -> [128, 64]
        if PROJ_PSUM == "w" and b == 0:
            tr_ps = pp.tile([P, 4, HD], BF16, tag="W", bufs=W_BUFS, name="trw")
        else:
            tr_ps = pp.tile([P, 4, HD], BF16, tag="PJ", bufs=PJ_BUFS, name="tr")
        for i in range(4):
            nc.tensor.transpose(
                tr_ps[:, i], vt_sb[:, i * P : (i + 1) * P], ident[:]
            )
        with band("proj"):
            nc.vector.tensor_copy(v1_sb[:, 4 * tb : 4 * tb + 4, 0:HD], tr_ps[:])

    def attn(st, qb):
        # ------------- attention scores + exp for q-block qb -----------
        b = st["b"]
        qt8_sb, khl_sb = st["qt8"], st["khl"]
        nkt = 4 * (qb + 1)  # causal: kpos tiles 0..nkt-1
        p4 = p_pool.tile([P, 2, 2, nkt, TB], BF16, tag=f"P{qb % 2}", bufs=1)
        with band("attn"):
            for kt in range(nkt):
                for pair in range(2):
                    p_sb = p4[:, pair]
                    ks = slice(kt * P, (kt + 1) * P)
                    dk = kt - qb * 4
                    off = max(dk, 0) * P  # first potentially-valid column
                    s_ps = pp.tile([P, 2, TB], F32, tag="S", bufs=S_BUFS)
                    qs = slice(qb * TB + off, (qb + 1) * TB)
                    # one fp8 DR matmul per head: (kh,kl) stationary x
                    # (qh,qh) moving = k . qh, K exact; psum = 8*score.
                    # The moving (qh,qh) pair is a stride-0 broadcast AP.
                    for half in range(2):
                        hp = slice(HD * half, HD * (half + 1))
                        qmv = qt8_sb[hp, pair, qs].unsqueeze(1)
                        nc.tensor.matmul(
                            s_ps[:, half, off:],
                            khl_sb[hp, :, ks],
                            qmv.broadcast_to([HD, 2, qmv.shape[2]]),
                            start=True,
                            stop=True,
                            perf_mode=DR,
                        )
                    nc.scalar.activation(
                        p_sb[:, :, kt, off:], s_ps[:, :, off:], AF.Exp,
                        scale=SEXP,
                    )
                    if dk >= 0:  # diagonal block: causal triangle mask
                        (nc.gpsimd if MASK_POOL else nc.vector).tensor_mul(
                            p_sb[:, :, kt, off : off + P],
                            p_sb[:, :, kt, off : off + P],
                            tri[:],
                        )
        return p4

    def attn_b(st, qb, p4):
        b = st["b"]
        v1_sb = st["v1"]
        ot_sb = ot_pool.tile([P, 2, TB], BF16, tag="ot")
        ot8h = ot8_pool.tile([P, 2, TB], FP8, tag="oh")
        ot8l = ot8_pool.tile([P, 2, TB], FP8, tag="ol")
        # --- phase B: PV accumulation, normalize, transpose ---
        def bj(j):
            for pair in range(2):
                p_sb = p4[:, pair]
                ktn = qb * 4 + j + 1  # kpos tiles 0..ktn-1
                o_n = on_pool.tile([P, 2, HD], BF16, tag="on")
                for half in range(2):
                    o_ps = pp.tile([P, HD + 1], F32, tag="O", bufs=O_BUFS)
                    with band("pv") if PV_BAND else _null():
                        for kt in range(ktn):
                            nc.tensor.matmul(
                                o_ps[:],
                                p_sb[:, half, kt, j * P : (j + 1) * P],
                                v1_sb[:, kt, :],
                                start=(kt == 0),
                                stop=(kt == ktn - 1),
                            )
                    rec = rc_pool.tile([P, 1], F32, tag="rec")
                    nc.vector.reciprocal(rec[:], o_ps[:, HD : HD + 1])
                    nc.vector.tensor_scalar(
                        o_n[:, half, :], o_ps[:, 0:HD], rec[:], 8.0,
                        mybir.AluOpType.mult, mybir.AluOpType.mult,
                    )
                js = slice(j * P, (j + 1) * P)
                if b == B - 1 and qb == 3 and j == 3:
                    # drain path: PE transpose instead of the xbar DMA
                    # transpose (~1.7 us latency); S is idle by then
                    tp = pp.tile(
                        [P, 2, TB], BF16, tag="S", bufs=S_BUFS,
                        name=f"tp{pair}",
                    )
                    nc.tensor.transpose(tp[:, 0, 0:P], o_n[:], ident128[:])
                    nc.vector.tensor_copy(ot_sb[:, pair, js], tp[:, 0, 0:P])
                else:
                    nc.sync.dma_start_transpose(ot_sb[:, pair, js], o_n[:])
                # SBUF-only fp8 split of ot (the piece of DVE work that
                # CAN move to Pool; GPSIMD cannot touch PSUM). The final
                # drain block stays on DVE for latency.
                drainb = DRAIN_OT8_DVE and b == B - 1 and qb == 3
                e8 = nc.gpsimd if (OT8_POOL and not drainb) else nc.vector
                e8.tensor_copy(ot8h[:, pair, js], ot_sb[:, pair, js])
                e8.tensor_sub(
                    ot8l[:, pair, js], ot_sb[:, pair, js], ot8h[:, pair, js]
                )
        return bj, (ot8h, ot8l)

    def _wo_j(st, qb, ots, j, split_stores=False):
        b = st["b"]
        ot8h, ot8l = ots
        rows = slice(qb * TB + j * P, qb * TB + (j + 1) * P)
        stg = stg_pool.tile([P, D], BF16, tag="stg")
        for nb in range(4):
            if b == 1 and qb == 2 and nb in (1, 3):
                wo_ps = pp.tile(
                    [P, TB], F32, tag="PJ", bufs=PJ_BUFS,
                    name=f"w2P{j}{nb}",
                )
            elif split_stores:
                # final block: every other psum tag is idle by now --
                # rotate through them so Wo is not W-recycle-bound
                if nb == 3 or nb == 1:
                    wo_ps = pp.tile(
                        [P, 2, TB], F32, tag="S", bufs=S_BUFS,
                        name=f"wS{j}{nb}",
                    )[:, 0]
                elif nb == 2:
                    wo_ps = pp.tile(
                        [P, TB], F32, tag="PJ", bufs=PJ_BUFS,
                        name=f"wP{j}",
                    )
                else:
                    wo_ps = pp.tile([P, TB], F32, tag="W", bufs=W_BUFS)
            else:
                wo_ps = pp.tile([P, TB], F32, tag="W", bufs=W_BUFS)
            ns = slice(nb * TB, (nb + 1) * TB)
            for i, (osb, wsb) in enumerate(
                ((ot8h, woh_sb), (ot8l, woh_sb), (ot8h, wol_sb))
            ):
                nc.tensor.matmul(
                    wo_ps[:],
                    osb[:, :, j * P : (j + 1) * P],
                    wsb[:, :, ns],
                    start=(i == 0),
                    stop=(i == 2),
                    perf_mode=DR,
                )
            if split_stores and (nb % 2 == 0 or not DRAIN_SPLIT):
                # final block: ACT is mostly idle (no exps left)
                nc.scalar.mul(stg[:, ns], wo_ps[:], 1.0 / (8.0 * WSCALE))
            else:
                nc.vector.tensor_scalar_mul(
                    stg[:, ns], wo_ps[:], 1.0 / (8.0 * WSCALE)
                )
            if split_stores:
                nc.sync.dma_start(
                    out[b, rows, nb * TB : (nb + 1) * TB],
                    stg[:, nb * TB : (nb + 1) * TB],
                )
        if not split_stores:
            nc.sync.dma_start(out[b, rows, :], stg[:])

    def ab_wo(st, qb, p4, split_stores=False, defer_wo_js=()):
        # Phase B + Wo, j-major so each row-block's Wo unlocks early.
        # defer_wo_js postpones those row-blocks' Wo to the returned thunk
        # (emitted later = lower priority = fills the exp-bound tail).
        bj, ots = attn_b(st, qb, p4)
        deferred = []
        for j in range(NT):
            bj(j)
            if j in defer_wo_js:
                deferred.append(j)
            else:
                _wo_j(st, qb, ots, j, split_stores)

        def run_deferred():
            for j in deferred:
                _wo_j(st, qb, ots, j, split_stores)

        return run_deferred

    # ---- global schedule. Emission order = scheduler priority. Rules:
    # * each proj is emitted BEFORE the neighboring ab_wo so its psum->SBUF
    #   copies (which gate the next attention block's scores) outrank the
    #   ab_wo normalize/Wo copies on the shared DVE queue;
    # * attn(qb+1) right before/after ab_wo(qb) as in the exp-stream
    #   pipeline; a P-tag is only reused after its readers are emitted;
    # * b1 projections fill b0's ACT-bound qb3 window; late Wo row-blocks
    #   are deferred into the underfilled b1-qb2/qb3 windows (each deferral
    #   lands before the ot8 buffer (bufs=2) it reads is recycled). ----
    s0 = batch_state(0)
    s1 = batch_state(1)
    if SCHED == "v7":
        proj(s0, 0)
        proj(s0, 1)
        p1 = attn(s0, 0)
        proj(s0, 2)
        p2 = attn(s0, 1)
        ab_wo(s0, 0, p1)
        proj(s0, 3)
        p3 = attn(s0, 2)
        ab_wo(s0, 1, p2)
        proj(s1, 0)
        p4_ = attn(s0, 3)
        d02 = ab_wo(s0, 2, p3, defer_wo_js=(0, 1, 2, 3))
        proj(s1, 1)
        proj(s1, 2)
        d03 = ab_wo(s0, 3, p4_, defer_wo_js=(0, 1, 2, 3))
        proj(s1, 3)
        q1 = attn(s1, 0)
        d02()
        q2 = attn(s1, 1)
        ab_wo(s1, 0, q1)
        d03()
        q3 = attn(s1, 2)
        ab_wo(s1, 1, q2)
        q4_ = attn(s1, 3)
        dfr = ab_wo(s1, 2, q3, defer_wo_js=(2, 3) if DEFER else ())
        dfr()
        ab_wo(s1, 3, q4_, split_stores=True)
    elif SCHED == "v6":
        proj(s0, 0)
        proj(s0, 1)
        p1 = attn(s0, 0)
        proj(s0, 2)
        p2 = attn(s0, 1)
        ab_wo(s0, 0, p1)
        proj(s0, 3)
        p3 = attn(s0, 2)
        d01 = ab_wo(s0, 1, p2, defer_wo_js=(0, 1, 2, 3))
        proj(s1, 0)
        d01()
        p4_ = attn(s0, 3)
        d02 = ab_wo(s0, 2, p3, defer_wo_js=(0, 1, 2, 3))
        proj(s1, 1)
        d02()
        proj(s1, 2)
        d03 = ab_wo(s0, 3, p4_, defer_wo_js=(0, 1, 2, 3))
        proj(s1, 3)
        d03()
        q1 = attn(s1, 0)
        q2 = attn(s1, 1)
        ab_wo(s1, 0, q1)
        q3 = attn(s1, 2)
        ab_wo(s1, 1, q2)
        q4_ = attn(s1, 3)
        dfr = ab_wo(s1, 2, q3, defer_wo_js=(2, 3) if DEFER else ())
        dfr()
        ab_wo(s1, 3, q4_, split_stores=True)
    elif SCHED == "v5":
        proj(s0, 0)
        proj(s0, 1)
        p1 = attn(s0, 0)
        proj(s0, 2)
        p2 = attn(s0, 1)
        ab_wo(s0, 0, p1)
        proj(s0, 3)
        p3 = attn(s0, 2)
        ab_wo(s0, 1, p2)
        proj(s1, 0)
        p4_ = attn(s0, 3)
        ab_wo(s0, 2, p3)
        proj(s1, 1)
        d03 = ab_wo(s0, 3, p4_, defer_wo_js=(1, 2, 3))
        proj(s1, 2)
        proj(s1, 3)
        q1 = attn(s1, 0)
        q2 = attn(s1, 1)
        d10 = ab_wo(s1, 0, q1, defer_wo_js=(2, 3))
        d03()
        q3 = attn(s1, 2)
        d11 = ab_wo(s1, 1, q2, defer_wo_js=(2, 3))
        d10()
        q4_ = attn(s1, 3)
        d12 = ab_wo(s1, 2, q3, defer_wo_js=(2, 3))
        d11()
        d12()
        ab_wo(s1, 3, q4_, split_stores=True)
    elif SCHED == "v4":
        proj(s0, 0)
        proj(s0, 1)
        p1 = attn(s0, 0)
        proj(s0, 2)
        p2 = attn(s0, 1)
        ab_wo(s0, 0, p1)
        proj(s0, 3)
        p3 = attn(s0, 2)
        proj(s1, 0)
        ab_wo(s0, 1, p2)
        proj(s1, 1)
        p4_ = attn(s0, 3)
        ab_wo(s0, 2, p3)
        proj(s1, 2)
        proj(s1, 3)
        ab_wo(s0, 3, p4_)
        q1 = attn(s1, 0)
        q2 = attn(s1, 1)
        ab_wo(s1, 0, q1)
        q3 = attn(s1, 2)
        ab_wo(s1, 1, q2)
        q4_ = attn(s1, 3)
        dfr = ab_wo(s1, 2, q3, defer_wo_js=(2, 3) if DEFER else ())
        dfr()
        ab_wo(s1, 3, q4_, split_stores=True)
    elif SCHED == "v1":
        proj(s0, 0)
        proj(s0, 1)
        p1 = attn(s0, 0)
        proj(s0, 2)
        p2 = attn(s0, 1)
        ab_wo(s0, 0, p1)
        proj(s0, 3)
        p3 = attn(s0, 2)
        ab_wo(s0, 1, p2)
        proj(s1, 0)
        p4_ = attn(s0, 3)
        ab_wo(s0, 2, p3)
        proj(s1, 1)
        ab_wo(s0, 3, p4_)
        proj(s1, 2)
        proj(s1, 3)
        q1 = attn(s1, 0)
        q2 = attn(s1, 1)
        ab_wo(s1, 0, q1)
        q3 = attn(s1, 2)
        ab_wo(s1, 1, q2)
        q4_ = attn(s1, 3)
        dfr = ab_wo(s1, 2, q3, defer_wo_js=(2, 3) if DEFER else ())
        dfr()
        ab_wo(s1, 3, q4_, split_stores=True)
    else:  # v3
        proj(s0, 0)
        p1 = attn(s0, 0)
        proj(s0, 1)
        p2 = attn(s0, 1)
        proj(s0, 2)
        ab_wo(s0, 0, p1)
        p3 = attn(s0, 2)
        proj(s0, 3)
        ab_wo(s0, 1, p2)
        proj(s1, 0)
        p4_ = attn(s0, 3)
        proj(s1, 1)
        ab_wo(s0, 2, p3)
        proj(s1, 2)
        q1 = attn(s1, 0)
        d03 = ab_wo(s0, 3, p4_, defer_wo_js=(2, 3))
        q2 = attn(s1, 1)
        proj(s1, 3)
        d10 = ab_wo(s1, 0, q1, defer_wo_js=(2, 3))
        d03()
        q3 = attn(s1, 2)
        d11 = ab_wo(s1, 1, q2, defer_wo_js=(1, 2, 3))
        d10()
        q4_ = attn(s1, 3)
        d12 = ab_wo(s1, 2, q3, defer_wo_js=(1, 2, 3))
        d11()
        ab_wo(s1, 3, q4_, split_stores=True)
        d12()


_NC_CACHE = {}


def get_nc():
    if "nc" not in _NC_CACHE:
        nc = bacc.Bacc("TRN2", target_bir_lowering=False, debug=False)
        with tile.TileContext(nc) as tc, ExitStack() as ctx:
            build_kernel(ctx, tc)
        nc.compile()
        _NC_CACHE["nc"] = nc
    return _NC_CACHE["nc"]


def make_in_maps(x, Wq, Wk, Wv, Wo):
    FP8NP = ml_dtypes.float8_e4m3

    def fp8_split(a):
        hi = a.astype(FP8NP)
        lo = (a - hi.astype(np.float32)).astype(FP8NP)
        return hi, lo

    xT = np.ascontiguousarray(np.transpose(np.asarray(x, np.float32), (0, 2, 1)))
    xh, xl = fp8_split(xT)
    Wq, Wk, Wv, Wo = (np.asarray(w, np.float32) for w in (Wq, Wk, Wv, Wo))
    in_maps = []
    for g in range(NCORES):
        in_maps.append(
            {
                "xh": xh,
                "xl": xl,
                **dict(
                    zip(
                        ("wqh", "wql"),
                        fp8_split(
                            WSCALE * np.ascontiguousarray(Wq[:, g * DQ : (g + 1) * DQ])
                        ),
                    )
                ),
                **dict(
                    zip(
                        ("wkvh", "wkvl"),
                        fp8_split(
                            np.ascontiguousarray(
                                np.concatenate(
                                    [
                                        WSCALE_K * Wk[:, g * HD : (g + 1) * HD],
                                        WSCALE * Wv[:, g * HD : (g + 1) * HD],
                                    ],
                                    axis=1,
                                )
                            )
                        ),
                    )
                ),
                **dict(
                    zip(
                        ("woh", "wol"),
                        fp8_split(
                            WSCALE
                            * np.ascontiguousarray(Wo[g * DQ : (g + 1) * DQ, :])
                        ),
                    )
                ),
            }
        )
    return in_maps


def run(x, Wq, Wk, Wv, Wo, trace=False):
    nc = get_nc()
    in_maps = make_in_maps(x, Wq, Wk, Wv, Wo)
    res = run_bass_kernel_spmd(nc, in_maps, list(range(NCORES)), trace=trace)
    acc = np.zeros((B, T, D), np.float32)
    for r in res.results:
        acc += np.asarray(r["out"], dtype=np.float32)
    return acc, res


def kernel(x, Wq, Wk, Wv, Wo):
    return run(x, Wq, Wk, Wv, Wo)[0]
